# revision 1
# baseline (speedup 1.0000x reference)
"""Bass/Trainium2 kernel for nn_Encoders_6751688590031.

4-layer transformer encoder, d_model=64, H=8 heads, L=1024, dff=256, B=8.
Sharding: data-parallel over batch across 8 NeuronCores (1 batch element
per core); weights replicated. The global jnp.max(w) softmax normalization
needs one tiny AllReduce(max) per layer.

Math notes (vs reference.py):
 - two_d_softm max-subtraction is skipped: per-(b,h) max only rescales
   e and s identically, and logits stay within +-40 so exp() cannot
   overflow fp32. The per-head max of e (needed for the global max of w:
   max w = nz * max(e) / s) is computed by a DVE reduce instead.
 - key-padding (-1e9) is folded into the logits matmul via an augmented
   contraction row: lhsT rows = [q_h (8 rows); ones], rhs rows =
   [k_h (8 rows); padneg], padneg[k] = -1e9 on padded keys.
 - k is not multiplied by keepf (exp(-1e9)==0 makes it redundant).
 - attn final scale nz/(s_h*G) is folded into the attn^T PSUM evacuation.
"""

import os
import sys

import numpy as np

for _p in (
    "/root/.axon_site",
    "/root/.axon_site/_ro/trn_rl_repo",
    "/root/.axon_site/_ro/pypackages",
    "/opt/trn_rl_repo",
):
    if os.path.isdir(_p) and _p not in sys.path:
        sys.path.append(_p)

import concourse.bass as bass
import concourse.bacc as bacc
import concourse.tile as tile
from concourse import bass_isa, mybir

F32 = mybir.dt.float32
F32R = mybir.dt.float32r
BF16 = mybir.dt.bfloat16

L = 1024          # sequence length
D = 64            # d_model
H = 8             # heads
DH = 8            # head dim
DFF = 256
NL = 4            # layers
P = 128           # partitions per token tile
NT = L // P       # 8 token tiles
AUG = 128         # 3 tiles x up-to-3 heads at 32-row pitch; rows 32m..+7 feats, 32m+8 ones/padneg
NQT = 3           # q/k head-group tiles (heads 0-2, 3-5, 6-7)
HPT = (3, 3, 2)   # heads per tile
NCORES = 8
NEG_BIG = 1.0e9
LN_EPS = 1e-9

Act = mybir.ActivationFunctionType
Alu = mybir.AluOpType


def _r(ap):
    """View an fp32 AP as float32r so the PE streams 1 col/cycle."""
    return ap.bitcast(F32R)


def build_bass():
    nc = bacc.Bacc(
        "TRN2", target_bir_lowering=False, debug=False, num_devices=NCORES
    )

    # ---- parameters (per core) ----
    x_in = nc.declare_dram_parameter("x", [L, D], F32, isOutput=False)
    keepf = nc.declare_dram_parameter("keepf", [L], F32, isOutput=False)
    padneg = nc.declare_dram_parameter("padneg", [L], F32, isOutput=False)
    Wq = nc.declare_dram_parameter("Wq", [NL, NQT, D, AUG], F32, isOutput=False)
    Wk = nc.declare_dram_parameter("Wk", [NL, NQT, D, AUG], F32, isOutput=False)
    Wv = nc.declare_dram_parameter("Wv", [NL, D, D], F32, isOutput=False)
    Wo = nc.declare_dram_parameter("Wo", [NL, D, D], F32, isOutput=False)
    bq_aug = nc.declare_dram_parameter("bq_aug", [NL, NQT, AUG], F32, isOutput=False)
    bk_aug = nc.declare_dram_parameter("bk_aug", [NL, NQT, AUG], F32, isOutput=False)
    bv = nc.declare_dram_parameter("bv", [NL, D], F32, isOutput=False)
    bo = nc.declare_dram_parameter("bo", [NL, D], F32, isOutput=False)
    W1 = nc.declare_dram_parameter("W1", [NL, D, DFF], F32, isOutput=False)
    b1 = nc.declare_dram_parameter("b1", [NL, DFF], F32, isOutput=False)
    W2 = nc.declare_dram_parameter("W2", [NL, DFF, D], F32, isOutput=False)
    b2 = nc.declare_dram_parameter("b2", [NL, D], F32, isOutput=False)
    g1 = nc.declare_dram_parameter("g1", [NL, D], F32, isOutput=False)
    be1 = nc.declare_dram_parameter("be1", [NL, D], F32, isOutput=False)
    g2 = nc.declare_dram_parameter("g2", [NL, D], F32, isOutput=False)
    be2 = nc.declare_dram_parameter("be2", [NL, D], F32, isOutput=False)
    out = nc.declare_dram_parameter("out", [L, D], F32, isOutput=True)

    dma = nc.sync.dma_start

    with tile.TileContext(nc) as tc:
        with (
            tc.tile_pool(name="const", bufs=1) as constp,
            tc.tile_pool(name="wpool", bufs=1) as wp,
            tc.tile_pool(name="persist", bufs=1) as pers,
            tc.tile_pool(name="acts", bufs=2) as acts,
            tc.tile_pool(name="epool", bufs=2) as epool,
            tc.tile_pool(name="stats", bufs=2) as stats,
            tc.tile_pool(name="psL", bufs=2, space="PSUM") as psL,
            tc.tile_pool(name="psA", bufs=2, space="PSUM") as psA,
            tc.tile_pool(name="psM", bufs=2, space="PSUM") as psM,
            tc.tile_pool(name="dram", bufs=1, space="DRAM") as dramp,
        ):
            # ================= constants =================
            ones_t = constp.tile([P, P], F32, name="ones_t")
            nc.vector.memset(ones_t, 1.0)
            I128 = constp.tile([P, P], F32, name="I128")
            nc.gpsimd.affine_select(
                out=I128, in_=ones_t, pattern=[[-1, P]],
                compare_op=Alu.is_equal, fill=0.0, base=0, channel_multiplier=1,
            )
            # E8T[h, 8h+d] = 1 : expands per-head scales [8,1] -> [64,1]
            # built as band-select: keep where 0 <= col - 8*row <= 7
            E8Ttmp = constp.tile([H, D], F32, name="E8Ttmp")
            nc.gpsimd.affine_select(
                out=E8Ttmp, in_=ones_t[0:H, 0:D], pattern=[[1, D]],
                compare_op=Alu.is_ge, fill=0.0, base=0, channel_multiplier=-DH,
            )
            E8T = constp.tile([H, D], F32, name="E8T")
            nc.gpsimd.affine_select(
                out=E8T, in_=E8Ttmp, pattern=[[-1, D]],
                compare_op=Alu.is_ge, fill=0.0, base=DH - 1, channel_multiplier=DH,
            )
            ones_row8 = constp.tile([1, H], F32, name="ones_row8")
            nc.vector.memset(ones_row8, 1.0)

            def bcast_ap(dram_vec, parts, inner):
                # [inner] DRAM vector -> [parts, inner] partition-broadcast AP
                return bass.AP(
                    tensor=dram_vec.tensor, offset=dram_vec.offset,
                    ap=[[0, parts]] + dram_vec.ap,
                )

            # keepf broadcast to all rows except the ones-rows {8,40,72},
            # which stay 1.0.  Segmented so every DMA has at most one wait.
            keep_b128 = constp.tile([AUG, L], F32, name="keep_b128")
            ones_roww = constp.tile([1, L], F32, name="ones_roww")
            nc.vector.memset(ones_roww, 1.0)
            segs = [(0, 8), (9, 40), (41, 72), (73, 128)]
            for a, b in segs:
                dma(out=keep_b128[a:b, :], in_=bcast_ap(keepf[:], b - a, L))
            for m in range(3):
                nc.gpsimd.dma_start(
                    out=keep_b128[32 * m + 8 : 32 * m + 9, :], in_=ones_roww[:])
            keep_b64 = constp.tile([D, L], F32, name="keep_b64")
            dma(out=keep_b64, in_=bcast_ap(keepf[:], D, L))
            # token-major keep: keep_all[p, j] = keepf[j*128+p]
            keep_JP = constp.tile([NT, P], F32, name="keep_JP")
            dma(out=keep_JP, in_=keepf.rearrange("(j p) -> j p", p=P))
            pKA = psM.tile([P, NT], F32, name="pKA", tag="m")
            nc.tensor.transpose(out=pKA, in_=keep_JP, identity=I128[0:NT, 0:NT])
            keep_all = constp.tile([P, NT], F32, name="keep_all")
            nc.vector.tensor_copy(keep_all, pKA)
            # keep_exp[p, (j,f)] = keepf[j*128+p] (DVE bcast along f)
            keep_exp = constp.tile([P, NT, D], F32, name="keep_exp")
            for j in range(NT):
                nc.vector.tensor_scalar(
                    out=keep_exp[:, j, :], in0=ones_t[:, 0:D],
                    scalar1=keep_all[:, j : j + 1], scalar2=None, op0=Alu.mult,
                )
            keep_exp2 = keep_exp.rearrange("p j f -> p (j f)")

            # nz = count of kept tokens -> [8,1] via PE partition-sum + bcast
            nzk = constp.tile([P, 1], F32, name="nzk")
            nc.vector.reduce_sum(out=nzk, in_=keep_all, axis=mybir.AxisListType.X)
            eps_c = constp.tile([P, 1], F32, name="eps_c")
            nc.vector.memset(eps_c, LN_EPS)
            pNZ = psM.tile([1, 1], F32, name="pNZ", tag="m")
            nc.tensor.matmul(pNZ, ones_t[:, 0:1], nzk)
            nz1 = constp.tile([1, 1], F32, name="nz1")
            nc.vector.tensor_copy(nz1, pNZ)
            pNZ8 = psM.tile([H, 1], F32, name="pNZ8", tag="m")
            nc.tensor.matmul(pNZ8, ones_row8, nz1)
            nz8 = constp.tile([H, 1], F32, name="nz8")
            nc.vector.tensor_copy(nz8, pNZ8)

            # ================= weights =================
            wq_t, wk_t, wv_t, wo_t = [], [], [], []
            w1a_t, w1b_t, w2a_t, w2b_t = [], [], [], []
            bqc_t, bkc_t, bvc_t, boc_t, b1c_t, b2c_t = [], [], [], [], [], []
            g1b_t, be1b_t, g2b_t, be2b_t = [], [], [], []
            for l in range(NL):
                wq = []
                wk = []
                for t in range(NQT):
                    wqx = wp.tile([D, AUG], F32R, name=f"wq{l}_{t}", tag=f"wq_{t}")
                    dma(out=wqx, in_=Wq[l, t].bitcast(F32R))
                    wq.append(wqx)
                    wkx = wp.tile([D, AUG], F32R, name=f"wk{l}_{t}", tag=f"wk_{t}")
                    dma(out=wkx, in_=Wk[l, t].bitcast(F32R))
                    wk.append(wkx)
                wv = wp.tile([D, D], F32R, name=f"wv{l}")
                dma(out=wv, in_=Wv[l].bitcast(F32R))
                wo = wp.tile([D, D], F32R, name=f"wo{l}")
                dma(out=wo, in_=Wo[l].bitcast(F32R))
                w1a = wp.tile([D, P], F32R, name=f"w1a{l}")
                dma(out=w1a, in_=W1[l][:, 0:P].bitcast(F32R))
                w1b = wp.tile([D, P], F32R, name=f"w1b{l}")
                dma(out=w1b, in_=W1[l][:, P:DFF].bitcast(F32R))
                w2a = wp.tile([P, D], F32R, name=f"w2a{l}")
                dma(out=w2a, in_=W2[l][0:P, :].bitcast(F32R))
                w2b = wp.tile([P, D], F32R, name=f"w2b{l}")
                dma(out=w2b, in_=W2[l][P:DFF, :].bitcast(F32R))
                bqc = []
                bkc = []
                for t in range(NQT):
                    bqx = wp.tile([AUG, 1], F32, name=f"bqc{l}_{t}", tag=f"bqc_{t}")
                    dma(out=bqx, in_=bq_aug[l, t].rearrange("(p o) -> p o", o=1))
                    bqc.append(bqx)
                    bkx = wp.tile([AUG, 1], F32, name=f"bkc{l}_{t}", tag=f"bkc_{t}")
                    dma(out=bkx, in_=bk_aug[l, t].rearrange("(p o) -> p o", o=1))
                    bkc.append(bkx)
                bvc = wp.tile([D, 1], F32, name=f"bvc{l}")
                dma(out=bvc, in_=bv[l].rearrange("(p o) -> p o", o=1))
                boc = wp.tile([D, 1], F32, name=f"boc{l}")
                dma(out=boc, in_=bo[l].rearrange("(p o) -> p o", o=1))
                b1c = wp.tile([P, 2], F32, name=f"b1c{l}")
                for i in range(2):
                    dma(out=b1c[:, i : i + 1],
                        in_=b1[l][i * P : (i + 1) * P].rearrange("(p o) -> p o", o=1))
                b2c = wp.tile([D, 1], F32, name=f"b2c{l}")
                dma(out=b2c, in_=b2[l].rearrange("(p o) -> p o", o=1))

                def ln_bcast(vec, name):
                    t = wp.tile([P, NT, D], F32, name=name)
                    dma(out=t, in_=bass.AP(tensor=vec.tensor, offset=vec.offset,
                                           ap=[[0, P], [0, NT]] + vec.ap))
                    return t.rearrange("p j f -> p (j f)")

                g1b_t.append(ln_bcast(g1[l], f"g1b{l}"))
                be1b_t.append(ln_bcast(be1[l], f"be1b{l}"))
                g2b_t.append(ln_bcast(g2[l], f"g2b{l}"))
                be2b_t.append(ln_bcast(be2[l], f"be2b{l}"))
                wq_t.append(wq); wk_t.append(wk); wv_t.append(wv); wo_t.append(wo)
                w1a_t.append(w1a); w1b_t.append(w1b); w2a_t.append(w2a); w2b_t.append(w2b)
                bqc_t.append(bqc); bkc_t.append(bkc); bvc_t.append(bvc)
                boc_t.append(boc); b1c_t.append(b1c); b2c_t.append(b2c)

            # qa/ka persist across layers: [t][32m+r, k] holds head 4t+m
            qaT = [pers.tile([AUG, L], F32R, name=f"qaT{t}") for t in range(NQT)]
            kaT = [pers.tile([AUG, L], F32R, name=f"kaT{t}") for t in range(NQT)]

            # ---- initial x load: token-major packed [128, (j,f)] ----
            x_all = acts.tile([P, NT * D], F32, name="x_all", tag="x")
            dma(out=x_all.rearrange("p (j f) -> p j f", f=D),
                in_=x_in.rearrange("(j p) f -> p j f", p=P))

            for l in range(NL):
                # ======== x^T (feature-major) ========
                xT = acts.tile([D, L], F32R, name=f"xT{l}", tag="xT")
                for half in range(2):
                    pX = psM.tile([D, 512], F32, name=f"pX{l}_{half}", tag="m")
                    for jj in range(4):
                        j = half * 4 + jj
                        nc.tensor.transpose(
                            out=pX[:, jj * P : (jj + 1) * P],
                            in_=x_all[:, j * D : (j + 1) * D], identity=I128,
                        )
                    nc.vector.tensor_copy(xT[:, half * 512 : (half + 1) * 512], pX)

                # ======== QKV projections ========
                for t in range(NQT):
                    pQ = psL.tile([AUG, L], F32, name=f"pQ{l}_{t}", tag="L")
                    for hf in range(2):
                        nc.tensor.matmul(pQ[:, hf * 512 : (hf + 1) * 512],
                                         wq_t[l][t],
                                         xT[:, hf * 512 : (hf + 1) * 512])
                    nc.vector.tensor_scalar(out=qaT[t], in0=pQ, scalar1=bqc_t[l][t],
                                            scalar2=None, op0=Alu.add)
                    nc.vector.tensor_mul(qaT[t], qaT[t], keep_b128)

                    pK = psL.tile([AUG, L], F32, name=f"pK{l}_{t}", tag="L")
                    for hf in range(2):
                        nc.tensor.matmul(pK[:, hf * 512 : (hf + 1) * 512],
                                         wk_t[l][t],
                                         xT[:, hf * 512 : (hf + 1) * 512])
                    nc.vector.tensor_scalar(out=kaT[t], in0=pK, scalar1=bkc_t[l][t],
                                            scalar2=None, op0=Alu.add)
                    for m in range(HPT[t]):  # padneg rows
                        nc.gpsimd.dma_start(
                            out=kaT[t][32 * m + 8 : 32 * m + 9, :],
                            in_=padneg.rearrange("(o n) -> o n", o=1).bitcast(F32R))

                pV = psL.tile([D, L], F32, name=f"pV{l}", tag="L")
                for hf in range(2):
                    nc.tensor.matmul(pV[:, hf * 512 : (hf + 1) * 512],
                                     wv_t[l], xT[:, hf * 512 : (hf + 1) * 512])
                vT = acts.tile([D, L], F32, name=f"vT{l}", tag="vT")
                nc.vector.tensor_scalar(out=vT, in0=pV, scalar1=bvc_t[l],
                                        scalar2=None, op0=Alu.add)
                nc.vector.tensor_mul(vT, vT, keep_b64)

                # v token-major bf16 [128, (j,d*8heads)] : cols j*64+hd
                pVt = psM.tile([P, 512], F32, name=f"pVt{l}", tag="m")
                for j in range(NT):
                    nc.tensor.transpose(
                        out=pVt[:, j * D : (j + 1) * D],
                        in_=vT[:, j * P : (j + 1) * P], identity=I128[0:D, 0:D],
                    )
                vt_all = acts.tile([P, 512], BF16, name=f"vt{l}", tag="vt")
                nc.vector.tensor_copy(vt_all, pVt)

                # ======== attention ========
                mx_s8 = stats.tile([P, H], F32, name=f"mxs{l}", tag="mxs")
                s_s8 = stats.tile([P, H], F32, name=f"ss{l}", tag="ss")
                attn_all = acts.tile([P, NT * D], F32, name=f"attn{l}", tag="attn")
                for h in range(H):
                    e_h = epool.tile([P, NT * L], BF16, name=f"e{l}_{h}", tag="e")
                    acc_h = stats.tile([P, NT], F32, name=f"acc{l}_{h}", tag="acc")
                    t, m = divmod(h, 3)
                    rb = 32 * m
                    for j in range(NT):
                        pL = psL.tile([P, L], F32, name=f"pL{l}_{h}_{j}", tag="L")
                        qa_s = qaT[t][rb : rb + 9, j * P : (j + 1) * P]
                        for hf in range(2):
                            nc.tensor.matmul(
                                pL[:, hf * 512 : (hf + 1) * 512], qa_s,
                                kaT[t][rb : rb + 9, hf * 512 : (hf + 1) * 512],
                            )
                        nc.scalar.activation(
                            out=e_h[:, j * L : (j + 1) * L], in_=pL, func=Act.Exp,
                            accum_out=acc_h[:, j : j + 1],
                        )
                    nc.vector.reduce_max(out=mx_s8[:, h : h + 1], in_=e_h,
                                         axis=mybir.AxisListType.X)
                    nc.vector.reduce_sum(out=s_s8[:, h : h + 1], in_=acc_h,
                                         axis=mybir.AxisListType.X)
                    # e^T v : out[k, d] accumulated over q tiles
                    pA = psA.tile([P, D], F32, name=f"pA{l}_{h}", tag="A")
                    for kt in range(NT):
                        for j in range(NT):
                            nc.tensor.matmul(
                                pA[:, kt * DH : (kt + 1) * DH],
                                e_h[:, j * L + kt * P : j * L + (kt + 1) * P],
                                vt_all[:, j * D + DH * h : j * D + DH * (h + 1)],
                                start=(j == 0), stop=(j == NT - 1),
                            )
                    nc.vector.tensor_copy(
                        attn_all.rearrange("p (kt hd) -> p kt hd", hd=D)[:, :, DH * h : DH * (h + 1)],
                        pA.rearrange("p (kt d) -> p kt d", d=DH),
                    )

                # ======== stats -> local max of w ========
                pS = psM.tile([H, P], F32, name=f"pS{l}", tag="m")
                nc.tensor.transpose(out=pS, in_=mx_s8, identity=I128)
                statTm = stats.tile([H, P], F32, name=f"statTm{l}", tag="statTm")
                nc.vector.tensor_copy(statTm, pS)
                pS2 = psM.tile([H, P], F32, name=f"pS2{l}", tag="m")
                nc.tensor.transpose(out=pS2, in_=s_s8, identity=I128)
                statTs = stats.tile([H, P], F32, name=f"statTs{l}", tag="statTs")
                nc.vector.tensor_copy(statTs, pS2)
                mxh8 = stats.tile([H, 1], F32, name=f"mxh{l}", tag="mxh")
                nc.vector.reduce_max(out=mxh8, in_=statTm, axis=mybir.AxisListType.X)
                sh8 = stats.tile([H, 1], F32, name=f"sh{l}", tag="sh")
                nc.vector.reduce_sum(out=sh8, in_=statTs, axis=mybir.AxisListType.X)
                rs8 = stats.tile([H, 1], F32, name=f"rs{l}", tag="rs")
                nc.vector.reciprocal(out=rs8, in_=sh8)
                t8 = stats.tile([H, 1], F32, name=f"t8{l}", tag="t8")
                nc.vector.tensor_mul(t8, mxh8, rs8)
                nc.vector.tensor_mul(t8, t8, nz8)
                pT = psM.tile([1, H], F32, name=f"pT{l}", tag="m")
                nc.tensor.transpose(out=pT, in_=t8, identity=I128[0:H, 0:H])
                t8row = stats.tile([1, H], F32, name=f"t8row{l}", tag="t8row")
                nc.vector.tensor_copy(t8row, pT)
                gl = stats.tile([1, 1], F32, name=f"gl{l}", tag="gl")
                nc.vector.reduce_max(out=gl, in_=t8row, axis=mybir.AxisListType.X)

                # ======== AllReduce(max) over cores ========
                cc_in = dramp.tile([1, 1], F32, name=f"cc_in{l}", tag=f"cc_in{l}")
                cc_out = dramp.tile([1, 1], F32, name=f"cc_out{l}",
                                    tag=f"cc_out{l}", addr_space="Shared")
                nc.gpsimd.dma_start(out=cc_in[:], in_=gl)
                nc.gpsimd.collective_compute(
                    "AllReduce", Alu.max,
                    replica_groups=[list(range(NCORES))],
                    ins=[cc_in.opt()], outs=[cc_out.opt()],
                )
                G = stats.tile([1, 1], F32, name=f"G{l}", tag=f"G{l}")
                nc.gpsimd.dma_start(out=G, in_=cc_out[:])

                # c64[8h+d] = nz / (s_h * G)
                pG8 = psM.tile([H, 1], F32, name=f"pG8{l}", tag="m")
                nc.tensor.matmul(pG8, ones_row8, G)
                rG8 = stats.tile([H, 1], F32, name=f"rG8{l}", tag="rG8")
                nc.vector.reciprocal(out=rG8, in_=pG8)
                c8 = stats.tile([H, 1], F32, name=f"c8{l}", tag="c8")
                nc.vector.tensor_mul(c8, rs8, rG8)
                nc.vector.tensor_mul(c8, c8, nz8)
                pC = psM.tile([D, 1], F32, name=f"pC{l}", tag="m")
                nc.tensor.matmul(pC, E8T, c8)
                c64 = stats.tile([D, 1], F32, name=f"c64{l}", tag="c64")
                nc.vector.tensor_copy(c64, pC)

                # ======== attn^T (scaled) + Wo ========
                attnT = acts.tile([D, L], F32R, name=f"attnT{l}", tag="attnT")
                for half in range(2):
                    pAT = psM.tile([D, 512], F32, name=f"pAT{l}_{half}", tag="m")
                    for kk in range(4):
                        kt = half * 4 + kk
                        nc.tensor.transpose(
                            out=pAT[:, kk * P : (kk + 1) * P],
                            in_=attn_all[:, kt * D : (kt + 1) * D],
                            identity=I128,
                        )
                    nc.vector.tensor_scalar(
                        out=attnT[:, half * 512 : (half + 1) * 512], in0=pAT,
                        scalar1=c64, scalar2=None, op0=Alu.mult,
                    )
                pWo = psL.tile([D, L], F32, name=f"pWo{l}", tag="L")
                for hf in range(2):
                    nc.tensor.matmul(pWo[:, hf * 512 : (hf + 1) * 512],
                                     wo_t[l], attnT[:, hf * 512 : (hf + 1) * 512])
                wo_out = acts.tile([D, L], F32, name=f"wo_out{l}", tag="wo_out")
                nc.vector.tensor_scalar(out=wo_out, in0=pWo, scalar1=boc_t[l],
                                        scalar2=None, op0=Alu.add)
                nc.vector.tensor_mul(wo_out, wo_out, keep_b64)

                # ======== residual + LN1 (token-major) ========
                def layernorm(src_T, res_all, gb, bb, x_out_name, x_out_tag):
                    # src_T [64, L] feature-major -> token-major, add residual,
                    # layernorm, scale/shift, mask. Returns [128, NT*D] tile.
                    pZ = psM.tile([P, 512], F32, name=x_out_name + "_pz", tag="m")
                    for j in range(NT):
                        nc.tensor.transpose(
                            out=pZ[:, j * D : (j + 1) * D],
                            in_=src_T[:, j * P : (j + 1) * P],
                            identity=I128[0:D, 0:D],
                        )
                    z_all = acts.tile([P, NT * D], F32, name=x_out_name + "_z", tag="z")
                    nc.vector.tensor_add(z_all, pZ, res_all)
                    bn6 = stats.tile([P, NT, 6], F32, name=x_out_name + "_bn6", tag="bn6")
                    mv = stats.tile([P, NT, 2], F32, name=x_out_name + "_mv", tag="mv")
                    for j in range(NT):
                        nc.vector.bn_stats(out=bn6[:, j, :], in_=z_all[:, j * D : (j + 1) * D])
                        nc.vector.bn_aggr(out=mv[:, j, :], in_=bn6[:, j, :])
                    rstd = stats.tile([P, NT], F32, name=x_out_name + "_rstd", tag="rstd")
                    vv = stats.tile([P, NT], F32, name=x_out_name + "_vv", tag="vv")
                    nc.vector.tensor_scalar(out=vv, in0=mv[:, :, 1], scalar1=LN_EPS,
                                            scalar2=None, op0=Alu.add)
                    # rsqrt: int bit-trick seed + 3 Newton iterations (DVE only)
                    iv = vv.bitcast(mybir.dt.int32)
                    ir = rstd.bitcast(mybir.dt.int32)
                    nc.vector.tensor_scalar(out=ir, in0=iv, scalar1=1, scalar2=None,
                                            op0=Alu.logical_shift_right)
                    nc.vector.tensor_scalar(out=ir, in0=ir, scalar1=-1, scalar2=0x5F3759DF,
                                            op0=Alu.mult, op1=Alu.add)
                    tq = stats.tile([P, NT], F32, name=x_out_name + "_tq", tag="tq")
                    for _ in range(3):
                        nc.vector.tensor_mul(tq, rstd, rstd)
                        nc.vector.tensor_mul(tq, tq, vv)
                        nc.vector.tensor_scalar(out=tq, in0=tq, scalar1=-0.5, scalar2=1.5,
                                                op0=Alu.mult, op1=Alu.add)
                        nc.vector.tensor_mul(rstd, rstd, tq)
                    o = acts.tile([P, NT * D], F32, name=x_out_name, tag=x_out_tag)
                    for j in range(NT):
                        nc.vector.tensor_scalar(
                            out=o[:, j * D : (j + 1) * D], in0=z_all[:, j * D : (j + 1) * D],
                            scalar1=mv[:, j, 0:1], scalar2=rstd[:, j : j + 1],
                            op0=Alu.subtract, op1=Alu.mult,
                        )
                    nc.vector.tensor_mul(o, o, gb)
                    nc.vector.tensor_add(o, o, bb)
                    nc.vector.tensor_mul(o, o, keep_exp2)
                    return o

                out1_all = layernorm(wo_out, x_all, g1b_t[l], be1b_t[l],
                                     f"out1_{l}", "out1")

                # ======== FFN ========
                out1T = acts.tile([D, L], F32R, name=f"out1T{l}", tag="out1T")
                for half in range(2):
                    pO = psM.tile([D, 512], F32, name=f"pO{l}_{half}", tag="m")
                    for jj in range(4):
                        j = half * 4 + jj
                        nc.tensor.transpose(
                            out=pO[:, jj * P : (jj + 1) * P],
                            in_=out1_all[:, j * D : (j + 1) * D], identity=I128,
                        )
                    nc.vector.tensor_copy(out1T[:, half * 512 : (half + 1) * 512], pO)

                h1 = []
                for i, w1x in enumerate((w1a_t[l], w1b_t[l])):
                    pH = psL.tile([P, L], F32, name=f"pH{l}_{i}", tag="L")
                    for hf in range(2):
                        nc.tensor.matmul(pH[:, hf * 512 : (hf + 1) * 512],
                                         w1x, out1T[:, hf * 512 : (hf + 1) * 512])
                    h1x = acts.tile([P, L], F32R, name=f"h1_{l}_{i}", tag=f"h1_{i}")
                    nc.vector.tensor_scalar(
                        out=h1x, in0=pH, scalar1=b1c_t[l][:, i : i + 1], scalar2=0.0,
                        op0=Alu.add, op1=Alu.max,
                    )
                    h1.append(h1x)

                pW2 = psL.tile([D, L], F32, name=f"pW2{l}", tag="L")
                for hf in range(2):
                    sl = slice(hf * 512, (hf + 1) * 512)
                    nc.tensor.matmul(pW2[:, sl], w2a_t[l], h1[0][:, sl],
                                     start=True, stop=False)
                    nc.tensor.matmul(pW2[:, sl], w2b_t[l], h1[1][:, sl],
                                     start=False, stop=True)
                ffnT = acts.tile([D, L], F32, name=f"ffnT{l}", tag="ffnT")
                nc.vector.tensor_scalar(out=ffnT, in0=pW2, scalar1=b2c_t[l],
                                        scalar2=None, op0=Alu.add)
                nc.vector.tensor_mul(ffnT, ffnT, keep_b64)

                x_all = layernorm(ffnT, out1_all, g2b_t[l], be2b_t[l],
                                  f"x_next_{l}", "x")

            dma(out=out.rearrange("(j p) f -> p j f", p=P),
                in_=x_all.rearrange("p (j f) -> p j f", f=D))

    return nc


_NC_CACHE = None


def _get_nc():
    global _NC_CACHE
    if _NC_CACHE is None:
        _NC_CACHE = build_bass()
    return _NC_CACHE


def _make_in_maps(inputs):
    x = np.asarray(inputs["x"], np.float32)
    protok = np.asarray(inputs["protok"])
    B = x.shape[0]
    keep = (protok != 0).astype(np.float32)
    padneg = (keep - 1.0) * NEG_BIG  # -1e9 at padded keys, 0 elsewhere

    bq = np.asarray(inputs["bq"], np.float32)
    bk = np.asarray(inputs["bk"], np.float32)
    bq_aug = np.zeros((NL, NQT, AUG), np.float32)
    bk_aug = np.zeros((NL, NQT, AUG), np.float32)
    for t in range(NQT):
        nh = HPT[t]
        idx = np.arange(8 * nh)
        feat_rows = 32 * (idx // DH) + (idx % DH)
        c0 = 8 * (3 * t)
        bq_aug[:, t, feat_rows] = bq[:, c0 : c0 + 8 * nh]
        bk_aug[:, t, feat_rows] = bk[:, c0 : c0 + 8 * nh]
        bq_aug[:, t, 32 * np.arange(nh) + 8] = 1.0  # ones row


    Wq_in = np.asarray(inputs["Wq"], np.float32)
    Wk_in = np.asarray(inputs["Wk"], np.float32)
    Wq_pad = np.zeros((NL, NQT, D, AUG), np.float32)
    Wk_pad = np.zeros((NL, NQT, D, AUG), np.float32)
    for t in range(NQT):
        for m in range(HPT[t]):
            h = 3 * t + m
            Wq_pad[:, t, :, 32 * m : 32 * m + DH] = Wq_in[:, :, DH * h : DH * (h + 1)]
            Wk_pad[:, t, :, 32 * m : 32 * m + DH] = Wk_in[:, :, DH * h : DH * (h + 1)]
    shared = dict(
        Wq=Wq_pad,
        Wk=Wk_pad,
        Wv=np.ascontiguousarray(inputs["Wv"], np.float32),
        Wo=np.ascontiguousarray(inputs["Wo"], np.float32),
        bq_aug=bq_aug, bk_aug=bk_aug,
        bv=np.ascontiguousarray(inputs["bv"], np.float32),
        bo=np.ascontiguousarray(inputs["bo"], np.float32),
        W1=np.ascontiguousarray(inputs["W1"], np.float32),
        b1=np.ascontiguousarray(inputs["b1"], np.float32),
        W2=np.ascontiguousarray(inputs["W2"], np.float32),
        b2=np.ascontiguousarray(inputs["b2"], np.float32),
        g1=np.ascontiguousarray(inputs["g1"], np.float32),
        be1=np.ascontiguousarray(inputs["be1"], np.float32),
        g2=np.ascontiguousarray(inputs["g2"], np.float32),
        be2=np.ascontiguousarray(inputs["be2"], np.float32),
    )
    in_maps = []
    for i in range(NCORES):
        b = i % B
        in_maps.append(dict(
            x=np.ascontiguousarray(x[b]),
            keepf=np.ascontiguousarray(keep[b]),
            padneg=np.ascontiguousarray(padneg[b]),
            **shared,
        ))
    return in_maps


def run_on_hw(inputs, trace=False, **kwargs):
    from concourse.bass_utils import run_bass_kernel_spmd

    nc = _get_nc()
    if not nc.is_finalized():
        nc.finalize()
    in_maps = _make_in_maps(inputs)
    res = run_bass_kernel_spmd(nc, in_maps, list(range(NCORES)), trace=trace, **kwargs)
    outs = np.stack([res.results[i]["out"] for i in range(NCORES)], axis=0)
    return outs.astype(np.float32), res


def kernel(**inputs):
    outs, _ = run_on_hw(inputs, trace=False)
    return outs



# revision 5
# speedup vs baseline: 1.3108x; 1.3108x over previous
"""Bass/Trainium2 kernel for nn_Encoders_6751688590031.

4-layer transformer encoder, d_model=64, H=8 heads, L=1024, dff=256, B=8.
Sharding: data-parallel over batch across 8 NeuronCores (1 batch element
per core); weights replicated. One AllReduce(max) per layer for the global
jnp.max(w) softmax normalization.

Design (v2, PE-tiled):
 - Logits: 4 heads packed per PE pass via 32-row tiling. qaT/kaT hold 4
   heads per group at 32-row pitch; rows 32m..32m+7 = head features,
   row 32m+8 = ones (q) / padneg (k). K=9 matmuls on row-tiles run
   concurrently.
 - e^T v: v-stationary col-tiled matmuls (M=9 incl. a ones column that
   accumulates per-head colsums -> s_h). Output lands feature-major as an
   augmented attnT [128, L]; Wo is applied via an augmented [128, 64]
   weight whose rows are scaled by c_h = nz/(s_h*G) before the matmul, so
   no separate attn scaling pass exists.
 - Biases folded into matmuls via augmented ones rows (xTaug/out1Taug).
 - Per-head max of e via running bf16 tensor_tensor max folds (2x DVE
   mode) + one final reduce; sums come from the ones column (exact fp32
   PSUM accumulation), so activation accumulators are not used.
 - exp() calls are [128, 1024] PSUM->SBUF bf16 with strided out AP.
 - keep-masks dropped where padding lanes are provably masked later.
"""

import os
import sys

import numpy as np

for _p in (
    "/root/.axon_site",
    "/root/.axon_site/_ro/trn_rl_repo",
    "/root/.axon_site/_ro/pypackages",
    "/opt/trn_rl_repo",
):
    if os.path.isdir(_p) and _p not in sys.path:
        sys.path.append(_p)

import concourse.bass as bass
import concourse.bacc as bacc
import concourse.tile as tile
from concourse import mybir

F32 = mybir.dt.float32
F32R = mybir.dt.float32r
BF16 = mybir.dt.bfloat16

L = 1024          # sequence length
D = 64            # d_model
H = 8             # heads
DH = 8            # head dim
DFF = 256
NL = 4            # layers
P = 128           # partitions per token tile
NT = L // P       # 8 token tiles
NG = 2            # head groups (4 heads each)
MPG = 4           # heads per group
NCORES = 8
NEG_BIG = 1.0e9
LN_EPS = 1e-9

Act = mybir.ActivationFunctionType
Alu = mybir.AluOpType


def _r(ap):
    return ap.bitcast(F32R)


def build_bass():
    nc = bacc.Bacc(
        "TRN2", target_bir_lowering=False, debug=False, num_devices=NCORES
    )

    x_in = nc.declare_dram_parameter("x", [L, D], F32, isOutput=False)
    keepf = nc.declare_dram_parameter("keepf", [L], F32, isOutput=False)
    padneg = nc.declare_dram_parameter("padneg", [L], F32, isOutput=False)
    WqA = nc.declare_dram_parameter("WqA", [NL, NG, D + 1, P], F32, isOutput=False)
    WkA = nc.declare_dram_parameter("WkA", [NL, NG, D + 1, P], F32, isOutput=False)
    WvA = nc.declare_dram_parameter("WvA", [NL, NG, D + 1, 36], F32, isOutput=False)
    WoA = nc.declare_dram_parameter("WoA", [NL, NG, P, D], F32, isOutput=False)
    bo = nc.declare_dram_parameter("bo", [NL, D], F32, isOutput=False)
    W1A = nc.declare_dram_parameter("W1A", [NL, 2, D + 1, P], F32, isOutput=False)
    W2 = nc.declare_dram_parameter("W2", [NL, 2, P, D], F32, isOutput=False)
    b2 = nc.declare_dram_parameter("b2", [NL, D], F32, isOutput=False)
    g1 = nc.declare_dram_parameter("g1", [NL, D], F32, isOutput=False)
    be1 = nc.declare_dram_parameter("be1", [NL, D], F32, isOutput=False)
    g2 = nc.declare_dram_parameter("g2", [NL, D], F32, isOutput=False)
    be2 = nc.declare_dram_parameter("be2", [NL, D], F32, isOutput=False)
    E32T = nc.declare_dram_parameter("E32T", [NG, H, P], F32, isOutput=False)
    SELS = nc.declare_dram_parameter("SELS", [NG, P, H], F32, isOutput=False)
    out = nc.declare_dram_parameter("out", [L, D], F32, isOutput=True)

    dma = nc.sync.dma_start

    with tile.TileContext(nc) as tc:
        with (
            tc.tile_pool(name="const", bufs=1) as constp,
            tc.tile_pool(name="wpool", bufs=1) as wp,
            tc.tile_pool(name="lnp", bufs=2) as lnp,
            tc.tile_pool(name="persist", bufs=1) as pers,
            tc.tile_pool(name="acts", bufs=2) as acts,
            tc.tile_pool(name="epool", bufs=3) as epool,
            tc.tile_pool(name="rpool", bufs=2) as rpool,
            tc.tile_pool(name="stats", bufs=2) as stats,
            tc.tile_pool(name="psL", bufs=3, space="PSUM") as psL,
            tc.tile_pool(name="psA", bufs=1, space="PSUM") as psA,
            tc.tile_pool(name="dram", bufs=1, space="DRAM") as dramp,
        ):
            # ================= constants =================
            ones_t = constp.tile([P, P], F32, name="ones_t")
            nc.vector.memset(ones_t, 1.0)
            I128 = constp.tile([P, P], F32, name="I128")
            nc.gpsimd.affine_select(
                out=I128, in_=ones_t, pattern=[[-1, P]],
                compare_op=Alu.is_equal, fill=0.0, base=0, channel_multiplier=1,
            )
            ones_row8 = constp.tile([1, H], F32, name="ones_row8")
            nc.vector.memset(ones_row8, 1.0)
            ones_roww = constp.tile([1, L], F32, name="ones_roww")
            nc.vector.memset(ones_roww, 1.0)

            def bcast_ap(dram_vec, parts, inner):
                return bass.AP(
                    tensor=dram_vec.tensor, offset=dram_vec.offset,
                    ap=[[0, parts]] + dram_vec.ap,
                )

            # keep broadcast [128, L]; rows 32m+8 stay 1.0 (ones rows)
            keep_b128 = constp.tile([P, L], F32, name="keep_b128")
            segs = [(0, 8), (9, 40), (41, 72), (73, 104), (105, 128)]
            for a, b in segs:
                dma(out=keep_b128[a:b, :], in_=bcast_ap(keepf[:], b - a, L))
            for m in range(MPG):
                nc.gpsimd.dma_start(
                    out=keep_b128[32 * m + 8: 32 * m + 9, :], in_=ones_roww[:])
            # keep broadcast [36, L] for v-aug; rows 9m+8 stay 1.0
            keep_b36 = constp.tile([36, L], F32, name="keep_b36")
            for m in range(MPG):
                dma(out=keep_b36[9 * m: 9 * m + 8, :], in_=bcast_ap(keepf[:], 8, L))
                nc.gpsimd.dma_start(
                    out=keep_b36[9 * m + 8: 9 * m + 9, :], in_=ones_roww[:])

            # token-major keep for nz and the final output mask
            keep_JP = constp.tile([NT, P], F32, name="keep_JP")
            dma(out=keep_JP, in_=keepf.rearrange("(j p) -> j p", p=P))
            pKA = psL.tile([P, L], F32, name="pKA", tag="L")
            nc.tensor.transpose(out=pKA[:, 0:NT], in_=keep_JP,
                                identity=I128[0:NT, 0:NT])
            keep_all = constp.tile([P, NT], F32, name="keep_all")
            nc.vector.tensor_copy(keep_all, pKA[:, 0:NT])
            keep_exp = constp.tile([P, NT, D], F32, name="keep_exp")
            for j in range(NT):
                nc.vector.tensor_scalar(
                    out=keep_exp[:, j, :], in0=ones_t[:, 0:D],
                    scalar1=keep_all[:, j: j + 1], scalar2=None, op0=Alu.mult,
                )
            keep_exp2 = keep_exp.rearrange("p j f -> p (j f)")

            # nz -> [8,1]
            nzk = constp.tile([P, 1], F32, name="nzk")
            nc.vector.reduce_sum(out=nzk, in_=keep_all, axis=mybir.AxisListType.X)
            pNZ = psL.tile([P, L], F32, name="pNZ", tag="L")
            nc.tensor.matmul(pNZ[0:1, 0:1], ones_t[:, 0:1], nzk)
            nz1 = constp.tile([1, 1], F32, name="nz1")
            nc.vector.tensor_copy(nz1, pNZ[0:1, 0:1])
            pNZ8 = psL.tile([P, L], F32, name="pNZ8", tag="L")
            nc.tensor.matmul(pNZ8[0:H, 0:1], ones_row8, nz1)
            nz8 = constp.tile([H, 1], F32, name="nz8")
            nc.vector.tensor_copy(nz8, pNZ8[0:H, 0:1])

            # head-expansion selectors
            e32t = []
            selt = []
            for g in range(NG):
                t = constp.tile([H, P], F32, name=f"e32t{g}")
                dma(out=t, in_=E32T[g])
                e32t.append(t)
                t2 = constp.tile([P, H], F32, name=f"selt{g}")
                dma(out=t2, in_=SELS[g])
                selt.append(t2)

            # ================= weights (all layers resident) =================
            wqa_t, wka_t, wva_t, woa_t = [], [], [], []
            w1a_t, w2_t = [], []
            boc_t, b2c_t = [], []
            for l in range(NL):
                wqa = []
                wka = []
                wva = []
                woa = []
                for g in range(NG):
                    t = wp.tile([D + 1, P], F32R, name=f"wqa{l}_{g}")
                    dma(out=t, in_=_r(WqA[l, g]))
                    wqa.append(t)
                    t = wp.tile([D + 1, P], F32R, name=f"wka{l}_{g}")
                    dma(out=t, in_=_r(WkA[l, g]))
                    wka.append(t)
                    t = wp.tile([D + 1, 36], F32R, name=f"wva{l}_{g}")
                    dma(out=t, in_=_r(WvA[l, g]))
                    wva.append(t)
                    t = wp.tile([P, D], F32, name=f"woa{l}_{g}")
                    dma(out=t, in_=WoA[l, g])
                    woa.append(t)
                w1a = []
                w2c = []
                for i in range(2):
                    t = wp.tile([D + 1, P], F32R, name=f"w1a{l}_{i}")
                    dma(out=t, in_=_r(W1A[l, i]))
                    w1a.append(t)
                    t = wp.tile([P, D], F32R, name=f"w2{l}_{i}")
                    dma(out=t, in_=_r(W2[l, i]))
                    w2c.append(t)
                boc = wp.tile([D, 1], F32, name=f"boc{l}")
                dma(out=boc, in_=bo[l].rearrange("(p o) -> p o", o=1))
                b2c = wp.tile([D, 1], F32, name=f"b2c{l}")
                dma(out=b2c, in_=b2[l].rearrange("(p o) -> p o", o=1))
                wqa_t.append(wqa); wka_t.append(wka); wva_t.append(wva)
                woa_t.append(woa); w1a_t.append(w1a); w2_t.append(w2c)
                boc_t.append(boc); b2c_t.append(b2c)

            def ln_bcast(vec, name):
                t = lnp.tile([P, NT, D], F32, name=name, tag=name.rstrip("0123456789"))
                dma(out=t, in_=bass.AP(tensor=vec.tensor, offset=vec.offset,
                                       ap=[[0, P], [0, NT]] + vec.ap))
                return t.rearrange("p j f -> p (j f)")

            # persistent augmented feature-major activations
            xTaug = pers.tile([D + 1, L], F32R, name="xTaug")
            nc.gpsimd.dma_start(out=xTaug[D: D + 1, :], in_=ones_roww[:].bitcast(F32R))
            o1Taug = pers.tile([D + 1, L], F32R, name="o1Taug")
            nc.gpsimd.dma_start(out=o1Taug[D: D + 1, :], in_=ones_roww[:].bitcast(F32R))
            qaT = [pers.tile([P, L], F32R, name=f"qaT{g}") for g in range(NG)]
            kaT = [pers.tile([P, L], F32R, name=f"kaT{g}") for g in range(NG)]

            # ---- initial x load: token-major packed [128, (j,f)] ----
            x_all = acts.tile([P, NT * D], F32, name="x_all", tag="x")
            dma(out=x_all.rearrange("p (j f) -> p j f", f=D),
                in_=x_in.rearrange("(j p) f -> p j f", p=P))

            for l in range(NL):
                g1b = ln_bcast(g1[l], f"g1b{l}")
                be1b = ln_bcast(be1[l], f"be1b{l}")
                g2b = ln_bcast(g2[l], f"g2b{l}")
                be2b = ln_bcast(be2[l], f"be2b{l}")

                # ======== xTaug (feature-major + ones row) ========
                pX = psA.tile([P, L], F32, name=f"pX{l}", tag="A")
                for j in range(NT):
                    nc.tensor.transpose(
                        out=pX[0:D, j * P: (j + 1) * P],
                        in_=x_all[:, j * D: (j + 1) * D], identity=I128,
                    )
                for half in range(2):
                    nc.vector.tensor_copy(
                        xTaug[0:D, half * 512: (half + 1) * 512],
                        pX[0:D, half * 512: (half + 1) * 512],
                    )

                # ======== QKV projections ========
                vta = []
                for g in range(NG):
                    pQ = psL.tile([P, L], F32, name=f"pQ{l}_{g}", tag="L")
                    for hf in range(2):
                        nc.tensor.matmul(pQ[:, hf * 512: (hf + 1) * 512],
                                         wqa_t[l][g],
                                         xTaug[:, hf * 512: (hf + 1) * 512])
                    nc.vector.tensor_tensor(qaT[g], pQ, keep_b128, op=Alu.mult)

                    pK = psL.tile([P, L], F32, name=f"pK{l}_{g}", tag="L")
                    for hf in range(2):
                        nc.tensor.matmul(pK[:, hf * 512: (hf + 1) * 512],
                                         wka_t[l][g],
                                         xTaug[:, hf * 512: (hf + 1) * 512])
                    nc.vector.tensor_copy(kaT[g], pK)
                    for m in range(MPG):
                        nc.gpsimd.dma_start(
                            out=kaT[g][32 * m + 8: 32 * m + 9, :],
                            in_=padneg.rearrange("(o n) -> o n", o=1).bitcast(F32R))

                    pV = psL.tile([P, L], F32, name=f"pV{l}_{g}", tag="L")
                    for hf in range(2):
                        nc.tensor.matmul(pV[0:36, hf * 512: (hf + 1) * 512],
                                         wva_t[l][g],
                                         xTaug[:, hf * 512: (hf + 1) * 512])
                    vTx = acts.tile([36, L], F32, name=f"vTx{l}_{g}", tag="vTx")
                    nc.vector.tensor_tensor(vTx, pV[0:36, :], keep_b36, op=Alu.mult)
                    # token-major v-aug bf16 [128, NT, 36]
                    pVt = psA.tile([P, L], F32, name=f"pVt{l}_{g}", tag="A")
                    for j in range(NT):
                        nc.tensor.transpose(
                            out=pVt[:, j * P: j * P + 36],
                            in_=vTx[:, j * P: (j + 1) * P],
                            identity=I128[0:36, 0:36],
                        )
                    vt_g = acts.tile([P, NT, 36], BF16, name=f"vta{l}_{g}",
                                     tag=f"vta{g}")
                    nc.vector.tensor_copy(
                        vt_g, pVt.rearrange("p (j c) -> p j c", c=P)[:, :, 0:36])
                    vta.append(vt_g)

                # ======== attention ========
                mxs = stats.tile([P, H], F32, name=f"mxs{l}", tag="mxs")
                attnTa = []
                sms = []
                for g in range(NG):
                    R_g = rpool.tile([P, MPG, L], BF16, name=f"R{l}_{g}", tag="R")
                    pAcc = psA.tile([P, L], F32, name=f"pAcc{l}_{g}", tag="A")
                    for j in range(NT):
                        et = epool.tile([P, MPG, L], BF16, name=f"e{l}_{g}_{j}",
                                        tag="e")
                        for kh in range(2):
                            for mp in range(2):
                                pL = psL.tile([P, L], F32,
                                              name=f"pL{l}_{g}_{j}_{kh}_{mp}",
                                              tag="L")
                                for mh in range(2):
                                    m = 2 * mp + mh
                                    rb = 32 * m
                                    nc.tensor.matmul(
                                        pL[:, mh * 512: (mh + 1) * 512],
                                        qaT[g][rb: rb + 9, j * P: (j + 1) * P],
                                        kaT[g][rb: rb + 9,
                                               kh * 512: (kh + 1) * 512],
                                        tile_position=(rb, 0),
                                    )
                                nc.scalar.activation(
                                    out=et[:, 2 * mp: 2 * mp + 2,
                                           kh * 512: (kh + 1) * 512],
                                    in_=pL.rearrange("p (m k) -> p m k", k=512),
                                    func=Act.Exp,
                                )
                        for kh in range(2):
                            for m in range(MPG):
                                nc.tensor.matmul(
                                    pAcc[32 * m: 32 * m + 9,
                                         kh * 512: (kh + 1) * 512],
                                    vta[g][:, j, 9 * m: 9 * m + 9],
                                    et[:, m, kh * 512: (kh + 1) * 512],
                                    start=(j == 0), stop=(j == NT - 1),
                                    skip_group_check=True,
                                    tile_position=(0, 32 * m),
                                )
                        if j == 0:
                            nc.vector.tensor_copy(R_g, et)
                        else:
                            nc.vector.tensor_tensor(R_g, R_g, et, op=Alu.max)
                    # per-head stats
                    nc.vector.tensor_reduce(
                        out=mxs[:, MPG * g: MPG * (g + 1)], in_=R_g,
                        axis=mybir.AxisListType.X, op=Alu.max,
                    )
                    sm = stats.tile([P, 1], F32, name=f"sms{l}_{g}", tag=f"sms{g}")
                    nc.vector.reduce_sum(out=sm, in_=pAcc,
                                         axis=mybir.AxisListType.X)
                    sms.append(sm)
                    at = acts.tile([P, L], F32R, name=f"attnTa{l}_{g}",
                                   tag=f"at{g}")
                    nc.vector.tensor_copy(at, pAcc)
                    attnTa.append(at)

                # ======== stats -> c8 ========
                pS = psL.tile([P, L], F32, name=f"pS{l}", tag="L")
                nc.tensor.transpose(out=pS[0:H, 0:P], in_=mxs, identity=I128)
                statT = stats.tile([H, P], F32, name=f"statT{l}", tag="statT")
                nc.vector.tensor_copy(statT, pS[0:H, 0:P])
                mx8 = stats.tile([H, 1], F32, name=f"mx8{l}", tag="mx8")
                nc.vector.reduce_max(out=mx8, in_=statT, axis=mybir.AxisListType.X)

                pS8 = psL.tile([P, L], F32, name=f"pS8{l}", tag="L")
                for g in range(NG):
                    nc.tensor.matmul(pS8[0:H, 0:1], selt[g], sms[g],
                                     start=(g == 0), stop=(g == NG - 1))
                s8 = stats.tile([H, 1], F32, name=f"s8{l}", tag="s8")
                nc.vector.tensor_copy(s8, pS8[0:H, 0:1])
                rs8 = stats.tile([H, 1], F32, name=f"rs8{l}", tag="rs8")
                nc.vector.reciprocal(out=rs8, in_=s8)
                t8 = stats.tile([H, 1], F32, name=f"t8{l}", tag="t8")
                nc.vector.tensor_mul(t8, mx8, rs8)
                nc.vector.tensor_mul(t8, t8, nz8)
                pT = psL.tile([P, L], F32, name=f"pT{l}", tag="L")
                nc.tensor.transpose(out=pT[0:1, 0:H], in_=t8,
                                    identity=I128[0:H, 0:H])
                t8row = stats.tile([1, H], F32, name=f"t8row{l}", tag="t8row")
                nc.vector.tensor_copy(t8row, pT[0:1, 0:H])
                gl = stats.tile([1, 1], F32, name=f"gl{l}", tag="gl")
                nc.vector.reduce_max(out=gl, in_=t8row, axis=mybir.AxisListType.X)

                # ======== AllReduce(max) over cores ========
                cc_in = dramp.tile([1, 1], F32, name=f"cc_in{l}", tag=f"cc_in{l}")
                cc_out = dramp.tile([1, 1], F32, name=f"cc_out{l}",
                                    tag=f"cc_out{l}", addr_space="Shared")
                nc.gpsimd.dma_start(out=cc_in[:], in_=gl)
                nc.gpsimd.collective_compute(
                    "AllReduce", Alu.max,
                    replica_groups=[list(range(NCORES))],
                    ins=[cc_in.opt()], outs=[cc_out.opt()],
                )
                G = stats.tile([1, 1], F32, name=f"G{l}", tag=f"G{l}")
                nc.gpsimd.dma_start(out=G, in_=cc_out[:])

                # c8 = nz / (s_h * G)
                pG8 = psL.tile([P, L], F32, name=f"pG8{l}", tag="L")
                nc.tensor.matmul(pG8[0:H, 0:1], ones_row8, G)
                rG8 = stats.tile([H, 1], F32, name=f"rG8{l}", tag="rG8")
                nc.vector.reciprocal(out=rG8, in_=pG8[0:H, 0:1])
                c8 = stats.tile([H, 1], F32, name=f"c8{l}", tag="c8")
                nc.vector.tensor_mul(c8, rs8, rG8)
                nc.vector.tensor_mul(c8, c8, nz8)

                # ======== Wo with c-scaled augmented weights ========
                pWo = psL.tile([P, L], F32, name=f"pWo{l}", tag="L")
                for g in range(NG):
                    pCa = psA.tile([P, L], F32, name=f"pCa{l}_{g}", tag="A")
                    nc.tensor.matmul(pCa[:, 0:1], e32t[g], c8)
                    ca = stats.tile([P, 1], F32, name=f"ca{l}_{g}", tag=f"ca{g}")
                    nc.vector.tensor_copy(ca, pCa[:, 0:1])
                    woS = stats.tile([P, D], F32R, name=f"woS{l}_{g}",
                                     tag=f"woS{g}")
                    nc.vector.tensor_scalar(out=woS, in0=woa_t[l][g],
                                            scalar1=ca, scalar2=None,
                                            op0=Alu.mult)
                    for hf in range(2):
                        nc.tensor.matmul(
                            pWo[0:D, hf * 512: (hf + 1) * 512],
                            woS, attnTa[g][:, hf * 512: (hf + 1) * 512],
                            start=(g == 0), stop=(g == NG - 1),
                            skip_group_check=True,
                        )
                wo_out = acts.tile([D, L], F32, name=f"wo_out{l}", tag="wo_out")
                nc.vector.tensor_scalar(out=wo_out, in0=pWo[0:D, :],
                                        scalar1=boc_t[l], scalar2=None,
                                        op0=Alu.add)

                # ======== residual + LN (token-major) ========
                def layernorm(src_T, res_all, gb, bb, x_out_name, x_out_tag,
                              mask=None):
                    pZ = psA.tile([P, L], F32, name=x_out_name + "_pz", tag="A")
                    for j in range(NT):
                        nc.tensor.transpose(
                            out=pZ[:, j * D: (j + 1) * D],
                            in_=src_T[:, j * P: (j + 1) * P],
                            identity=I128[0:D, 0:D],
                        )
                    z_all = acts.tile([P, NT * D], F32, name=x_out_name + "_z",
                                      tag="z")
                    nc.vector.tensor_tensor(z_all, pZ[:, 0: NT * D], res_all,
                                            op=Alu.add)
                    bn6 = stats.tile([P, NT, 6], F32, name=x_out_name + "_bn6",
                                     tag="bn6")
                    mv = stats.tile([P, NT, 2], F32, name=x_out_name + "_mv",
                                    tag="mv")
                    for j in range(NT):
                        nc.vector.bn_stats(out=bn6[:, j, :],
                                           in_=z_all[:, j * D: (j + 1) * D])
                        nc.vector.bn_aggr(out=mv[:, j, :], in_=bn6[:, j, :])
                    rstd = stats.tile([P, NT], F32, name=x_out_name + "_rstd",
                                      tag="rstd")
                    vv = stats.tile([P, NT], F32, name=x_out_name + "_vv",
                                    tag="vv")
                    nc.vector.tensor_scalar(out=vv, in0=mv[:, :, 1],
                                            scalar1=LN_EPS, scalar2=None,
                                            op0=Alu.add)
                    iv = vv.bitcast(mybir.dt.int32)
                    ir = rstd.bitcast(mybir.dt.int32)
                    nc.vector.tensor_scalar(out=ir, in0=iv, scalar1=1,
                                            scalar2=None,
                                            op0=Alu.logical_shift_right)
                    nc.vector.tensor_scalar(out=ir, in0=ir, scalar1=-1,
                                            scalar2=0x5F3759DF,
                                            op0=Alu.mult, op1=Alu.add)
                    tq = stats.tile([P, NT], F32, name=x_out_name + "_tq",
                                    tag="tq")
                    for _ in range(3):
                        nc.vector.tensor_mul(tq, rstd, rstd)
                        nc.vector.tensor_mul(tq, tq, vv)
                        nc.vector.tensor_scalar(out=tq, in0=tq, scalar1=-0.5,
                                                scalar2=1.5,
                                                op0=Alu.mult, op1=Alu.add)
                        nc.vector.tensor_mul(rstd, rstd, tq)
                    o = acts.tile([P, NT * D], F32, name=x_out_name,
                                  tag=x_out_tag)
                    for j in range(NT):
                        nc.vector.tensor_scalar(
                            out=o[:, j * D: (j + 1) * D],
                            in0=z_all[:, j * D: (j + 1) * D],
                            scalar1=mv[:, j, 0:1], scalar2=rstd[:, j: j + 1],
                            op0=Alu.subtract, op1=Alu.mult,
                        )
                    nc.vector.tensor_mul(o, o, gb)
                    nc.vector.tensor_add(o, o, bb)
                    if mask is not None:
                        nc.vector.tensor_mul(o, o, mask)
                    return o

                out1_all = layernorm(wo_out, x_all, g1b, be1b,
                                     f"out1_{l}", "out1")

                # ======== FFN ========
                pO = psA.tile([P, L], F32, name=f"pO{l}", tag="A")
                for j in range(NT):
                    nc.tensor.transpose(
                        out=pO[0:D, j * P: (j + 1) * P],
                        in_=out1_all[:, j * D: (j + 1) * D], identity=I128,
                    )
                for half in range(2):
                    nc.vector.tensor_copy(
                        o1Taug[0:D, half * 512: (half + 1) * 512],
                        pO[0:D, half * 512: (half + 1) * 512],
                    )

                h1 = []
                for i in range(2):
                    pH = psL.tile([P, L], F32, name=f"pH{l}_{i}", tag="L")
                    for hf in range(2):
                        nc.tensor.matmul(pH[:, hf * 512: (hf + 1) * 512],
                                         w1a_t[l][i],
                                         o1Taug[:, hf * 512: (hf + 1) * 512])
                    h1x = acts.tile([P, L], F32R, name=f"h1_{l}_{i}",
                                    tag=f"h1_{i}")
                    nc.scalar.activation(out=h1x, in_=pH, func=Act.Relu)
                    h1.append(h1x)

                pW2 = psL.tile([P, L], F32, name=f"pW2{l}", tag="L")
                for hf in range(2):
                    sl = slice(hf * 512, (hf + 1) * 512)
                    nc.tensor.matmul(pW2[0:D, sl], w2_t[l][0], h1[0][:, sl],
                                     start=True, stop=False)
                    nc.tensor.matmul(pW2[0:D, sl], w2_t[l][1], h1[1][:, sl],
                                     start=False, stop=True)
                ffnT = acts.tile([D, L], F32, name=f"ffnT{l}", tag="ffnT")
                nc.vector.tensor_scalar(out=ffnT, in0=pW2[0:D, :],
                                        scalar1=b2c_t[l], scalar2=None,
                                        op0=Alu.add)

                x_all = layernorm(ffnT, out1_all, g2b, be2b,
                                  f"x_next_{l}", "x",
                                  mask=keep_exp2 if l == NL - 1 else None)

            dma(out=out.rearrange("(j p) f -> p j f", p=P),
                in_=x_all.rearrange("p (j f) -> p j f", f=D))

    return nc


_NC_CACHE = None


def _get_nc():
    global _NC_CACHE
    if _NC_CACHE is None:
        _NC_CACHE = build_bass()
    return _NC_CACHE


def _make_in_maps(inputs):
    x = np.asarray(inputs["x"], np.float32)
    protok = np.asarray(inputs["protok"])
    B = x.shape[0]
    keep = (protok != 0).astype(np.float32)
    padneg = (keep - 1.0) * NEG_BIG

    Wq = np.asarray(inputs["Wq"], np.float32)
    Wk = np.asarray(inputs["Wk"], np.float32)
    Wv = np.asarray(inputs["Wv"], np.float32)
    Wo = np.asarray(inputs["Wo"], np.float32)
    bq = np.asarray(inputs["bq"], np.float32)
    bk = np.asarray(inputs["bk"], np.float32)
    bv = np.asarray(inputs["bv"], np.float32)
    W1 = np.asarray(inputs["W1"], np.float32)
    b1 = np.asarray(inputs["b1"], np.float32)
    W2in = np.asarray(inputs["W2"], np.float32)

    WqA = np.zeros((NL, NG, D + 1, P), np.float32)
    WkA = np.zeros((NL, NG, D + 1, P), np.float32)
    WvA = np.zeros((NL, NG, D + 1, 36), np.float32)
    WoA = np.zeros((NL, NG, P, D), np.float32)
    for g in range(NG):
        for m in range(MPG):
            h = MPG * g + m
            cols = slice(DH * h, DH * (h + 1))
            WqA[:, g, 0:D, 32 * m: 32 * m + DH] = Wq[:, :, cols]
            WqA[:, g, D, 32 * m: 32 * m + DH] = bq[:, cols]
            WqA[:, g, D, 32 * m + 8] = 1.0
            WkA[:, g, 0:D, 32 * m: 32 * m + DH] = Wk[:, :, cols]
            WkA[:, g, D, 32 * m: 32 * m + DH] = bk[:, cols]
            WvA[:, g, 0:D, 9 * m: 9 * m + DH] = Wv[:, :, cols]
            WvA[:, g, D, 9 * m: 9 * m + DH] = bv[:, cols]
            WvA[:, g, D, 9 * m + 8] = 1.0
            WoA[:, g, 32 * m: 32 * m + DH, :] = Wo[:, cols, :]

    W1A = np.zeros((NL, 2, D + 1, P), np.float32)
    for i in range(2):
        W1A[:, i, 0:D, :] = W1[:, :, i * P: (i + 1) * P]
        W1A[:, i, D, :] = b1[:, i * P: (i + 1) * P]
    W2A = np.ascontiguousarray(
        W2in.reshape(NL, 2, P, D))

    E32Tc = np.zeros((NG, H, P), np.float32)
    SELSc = np.zeros((NG, P, H), np.float32)
    for g in range(NG):
        for m in range(MPG):
            h = MPG * g + m
            E32Tc[g, h, 32 * m: 32 * m + DH] = 1.0
            SELSc[g, 32 * m + 8, h] = 1.0

    shared = dict(
        WqA=WqA, WkA=WkA, WvA=WvA, WoA=WoA,
        bo=np.ascontiguousarray(inputs["bo"], np.float32),
        W1A=W1A, W2=W2A,
        b2=np.ascontiguousarray(inputs["b2"], np.float32),
        g1=np.ascontiguousarray(inputs["g1"], np.float32),
        be1=np.ascontiguousarray(inputs["be1"], np.float32),
        g2=np.ascontiguousarray(inputs["g2"], np.float32),
        be2=np.ascontiguousarray(inputs["be2"], np.float32),
        E32T=E32Tc, SELS=SELSc,
    )
    in_maps = []
    for i in range(NCORES):
        b = i % B
        in_maps.append(dict(
            x=np.ascontiguousarray(x[b]),
            keepf=np.ascontiguousarray(keep[b]),
            padneg=np.ascontiguousarray(padneg[b]),
            **shared,
        ))
    return in_maps


def run_on_hw(inputs, trace=False, **kwargs):
    from concourse.bass_utils import run_bass_kernel_spmd

    nc = _get_nc()
    if not nc.is_finalized():
        nc.finalize()
    in_maps = _make_in_maps(inputs)
    res = run_bass_kernel_spmd(nc, in_maps, list(range(NCORES)), trace=trace, **kwargs)
    outs = np.stack([res.results[i]["out"] for i in range(NCORES)], axis=0)
    return outs.astype(np.float32), res


def kernel(**inputs):
    outs, _ = run_on_hw(inputs, trace=False)
    return outs


# revision 13
# speedup vs baseline: 1.3789x; 1.0520x over previous
"""Bass/Trainium2 kernel for nn_Encoders_6751688590031.

4-layer transformer encoder, d_model=64, H=8 heads, L=1024, dff=256, B=8.
Sharding: data-parallel over batch across 8 NeuronCores (1 batch element
per core); weights replicated. One AllReduce(max) per layer for the global
jnp.max(w) softmax normalization.

v3 design:
 - Logits: 4 heads packed per PE pass via 32-row tiling (K=9 row-tiles
   run concurrently). qaT/kaT hold 4 heads per group at 32-row pitch;
   row 32m+8 = ones (q) / padneg (k).
 - e^T v: v-stationary col-tiled matmuls (M=9 incl. ones column that
   accumulates per-head colsums -> s_h), PSUM-accumulated over q tiles.
   Output is an augmented feature-major attnT [128, L].
 - Wo and W2 applied TOKEN-major (lhsT = activation tile, rhs = weight),
   so no transposes / evacuations before the LNs; residual+bias adds are
   fused TTs from PSUM.
 - nz/s_h is folded into Wo's rhs before the AllReduce; the global 1/G
   is a post-AllReduce scalar fused into the residual add, so the Wo
   matmuls overlap the collective.
 - Per-head max of e: running bf16 TT-max folds (2x DVE) + final gpsimd
   XYZWC reduces (off the DVE critical path).
 - exp(): [128,1024] PSUM->SBUF bf16, strided out, 3-deep PSUM rotation.
 - Biases via augmented ones rows; grouped bn_stats; 2-step Newton rsqrt.
"""

import os
import sys

import numpy as np

for _p in (
    "/root/.axon_site",
    "/root/.axon_site/_ro/trn_rl_repo",
    "/root/.axon_site/_ro/pypackages",
    "/opt/trn_rl_repo",
):
    if os.path.isdir(_p) and _p not in sys.path:
        sys.path.append(_p)

import concourse.bass as bass
import concourse.bacc as bacc
import concourse.tile as tile
from concourse import mybir

F32 = mybir.dt.float32
F32R = mybir.dt.float32r
BF16 = mybir.dt.bfloat16

L = 1024          # sequence length
D = 64            # d_model
H = 8             # heads
DH = 8            # head dim
DFF = 256
NL = 4            # layers
P = 128           # partitions per token tile
NT = L // P       # 8 token tiles
NG = 2            # head groups (4 heads each)
MPG = 4           # heads per group
NCORES = 8
NEG_BIG = 1.0e9
LN_EPS = 1e-9

Act = mybir.ActivationFunctionType
Alu = mybir.AluOpType


def _r(ap):
    return ap.bitcast(F32R)


def build_bass():
    nc = bacc.Bacc(
        "TRN2", target_bir_lowering=False, debug=False, num_devices=NCORES
    )

    x_in = nc.declare_dram_parameter("x", [L, D], F32, isOutput=False)
    keepf = nc.declare_dram_parameter("keepf", [L], F32, isOutput=False)
    padneg = nc.declare_dram_parameter("padneg", [L], F32, isOutput=False)
    WqA = nc.declare_dram_parameter("WqA", [NL, NG, D + 1, P], F32, isOutput=False)
    WkA = nc.declare_dram_parameter("WkA", [NL, NG, D + 1, P], F32, isOutput=False)
    WvA = nc.declare_dram_parameter("WvA", [NL, NG, D + 1, 36], F32, isOutput=False)
    WoA = nc.declare_dram_parameter("WoA", [NL, NG, P, D], F32, isOutput=False)
    W1A = nc.declare_dram_parameter("W1A", [NL, 2, D + 1, P], F32, isOutput=False)
    W2 = nc.declare_dram_parameter("W2", [NL, 2, P, D], F32, isOutput=False)
    LNV = nc.declare_dram_parameter("LNV", [NL, 6, D], F32, isOutput=False)
    B32 = nc.declare_dram_parameter("B32", [P, P], F32, isOutput=False)
    SELS = nc.declare_dram_parameter("SELS", [NG, P, H], F32, isOutput=False)
    out = nc.declare_dram_parameter("out", [L, D], F32, isOutput=True)

    dma = nc.sync.dma_start

    with tile.TileContext(nc) as tc:
        with (
            tc.tile_pool(name="const", bufs=1) as constp,
            tc.tile_pool(name="wpool", bufs=1) as wp,
            tc.tile_pool(name="lnp", bufs=2) as lnp,
            tc.tile_pool(name="persist", bufs=1) as pers,
            tc.tile_pool(name="acts", bufs=2) as acts,
            tc.tile_pool(name="epool", bufs=3) as epool,
            tc.tile_pool(name="rpool", bufs=2) as rpool,
            tc.tile_pool(name="stats", bufs=2) as stats,
            tc.tile_pool(name="psL", bufs=3, space="PSUM") as psL,
            tc.tile_pool(name="psA", bufs=1, space="PSUM") as psA,
            tc.tile_pool(name="dram", bufs=1, space="DRAM") as dramp,
        ):
            # ================= constants =================
            ones_t = constp.tile([P, P], F32, name="ones_t")
            nc.vector.memset(ones_t, 1.0)
            I128 = constp.tile([P, P], F32, name="I128")
            nc.gpsimd.affine_select(
                out=I128, in_=ones_t, pattern=[[-1, P]],
                compare_op=Alu.is_equal, fill=0.0, base=0, channel_multiplier=1,
            )
            ones_row8 = constp.tile([1, H], F32, name="ones_row8")
            nc.vector.memset(ones_row8, 1.0)
            ones_roww = constp.tile([1, L], F32, name="ones_roww")
            nc.vector.memset(ones_roww, 1.0)

            # zero psA banks once: never-written quadrant rows of the e^T v
            # accumulator then stay finite forever after
            pZI = psA.tile([P, L], F32, name="pZI", tag="A")
            nc.vector.memset(pZI, 0.0)

            def bcast_ap(dram_vec, parts, inner):
                return bass.AP(
                    tensor=dram_vec.tensor, offset=dram_vec.offset,
                    ap=[[0, parts]] + dram_vec.ap,
                )

            # keep broadcast [128, L]; rows 32m+8 stay 1.0 (ones rows)
            keep_b128 = constp.tile([P, L], F32, name="keep_b128")
            segs = [(0, 8), (9, 40), (41, 72), (73, 104), (105, 128)]
            for a, b in segs:
                dma(out=keep_b128[a:b, :], in_=bcast_ap(keepf[:], b - a, L))
            for m in range(MPG):
                nc.gpsimd.dma_start(
                    out=keep_b128[32 * m + 8: 32 * m + 9, :], in_=ones_roww[:])
            # keep broadcast [36, L] for v-aug; rows 9m+8 stay 1.0
            keep_b36 = constp.tile([36, L], F32, name="keep_b36")
            for m in range(MPG):
                dma(out=keep_b36[9 * m: 9 * m + 8, :], in_=bcast_ap(keepf[:], 8, L))
                nc.gpsimd.dma_start(
                    out=keep_b36[9 * m + 8: 9 * m + 9, :], in_=ones_roww[:])

            # token-major keep for nz and the final output mask
            keep_JP = constp.tile([NT, P], F32, name="keep_JP")
            dma(out=keep_JP, in_=keepf.rearrange("(j p) -> j p", p=P))
            pKA = psL.tile([P, L], F32, name="pKA", tag="L")
            nc.tensor.transpose(out=pKA[:, 0:NT], in_=keep_JP,
                                identity=I128[0:NT, 0:NT])
            keep_all = constp.tile([P, NT], F32, name="keep_all")
            nc.vector.tensor_copy(keep_all, pKA[:, 0:NT])
            keep_exp = constp.tile([P, NT, D], F32, name="keep_exp")
            for j in range(NT):
                nc.vector.tensor_scalar(
                    out=keep_exp[:, j, :], in0=ones_t[:, 0:D],
                    scalar1=keep_all[:, j: j + 1], scalar2=None, op0=Alu.mult,
                )
            keep_exp2 = keep_exp.rearrange("p j f -> p (j f)")

            # nz scalars: nz1 [1,1], nz8row [1,8], nzc [128,1]
            nzk = constp.tile([P, 1], F32, name="nzk")
            nc.vector.reduce_sum(out=nzk, in_=keep_all, axis=mybir.AxisListType.X)
            pNZ = psL.tile([P, L], F32, name="pNZ", tag="L")
            nc.tensor.matmul(pNZ[0:1, 0:1], ones_t[:, 0:1], nzk)
            nz1 = constp.tile([1, 1], F32, name="nz1")
            nc.vector.tensor_copy(nz1, pNZ[0:1, 0:1])
            pNZ8 = psL.tile([P, L], F32, name="pNZ8", tag="L")
            nc.tensor.matmul(pNZ8[0:H, 0:1], ones_row8, nz1)
            nc.tensor.matmul(pNZ8[:, 2:3], ones_t[0:1, :], nz1)
            nz8 = constp.tile([H, 1], F32, name="nz8")
            nc.vector.tensor_copy(nz8, pNZ8[0:H, 0:1])
            nzc = constp.tile([P, 1], F32, name="nzc")
            nc.vector.tensor_copy(nzc, pNZ8[:, 2:3])

            b32t = constp.tile([P, P], F32, name="b32t")
            dma(out=b32t, in_=B32[:, :])
            selt = []
            for g in range(NG):
                t2 = constp.tile([P, H], F32, name=f"selt{g}")
                dma(out=t2, in_=SELS[g])
                selt.append(t2)

            # ================= weights (consolidated DMAs) =================
            wqa_all = wp.tile([D + 1, NL, NG, P], F32R, name="wqa_all")
            dma(out=wqa_all, in_=_r(WqA[:, :, :, :].rearrange("l g r c -> r l g c")))
            wka_all = wp.tile([D + 1, NL, NG, P], F32R, name="wka_all")
            dma(out=wka_all, in_=_r(WkA[:, :, :, :].rearrange("l g r c -> r l g c")))
            wva_all = wp.tile([D + 1, NL, NG, 36], F32R, name="wva_all")
            dma(out=wva_all, in_=_r(WvA[:, :, :, :].rearrange("l g r c -> r l g c")))
            woa_all = wp.tile([P, NL, NG, D], F32, name="woa_all")
            dma(out=woa_all, in_=WoA[:, :, :, :].rearrange("l g r c -> r l g c"))
            w1a_all = wp.tile([D + 1, NL, 2, P], F32R, name="w1a_all")
            dma(out=w1a_all, in_=_r(W1A[:, :, :, :].rearrange("l i r c -> r l i c")))
            w2_all = wp.tile([P, NL, 2, D], F32R, name="w2_all")
            dma(out=w2_all, in_=_r(W2[:, :, :, :].rearrange("l i r c -> r l i c")))

            # persistent augmented feature-major activations
            xTaug = pers.tile([D + 1, L], F32R, name="xTaug")
            nc.gpsimd.dma_start(out=xTaug[D: D + 1, :],
                                in_=ones_roww[:].bitcast(F32R))
            o1Taug = pers.tile([D + 1, L], F32R, name="o1Taug")
            nc.gpsimd.dma_start(out=o1Taug[D: D + 1, :],
                                in_=ones_roww[:].bitcast(F32R))
            qaT = [pers.tile([P, L], F32R, name=f"qaT{g}") for g in range(NG)]
            kaT = [pers.tile([P, L], F32R, name=f"kaT{g}") for g in range(NG)]

            # ---- initial x load: token-major packed [128, (j,f)] ----
            x_all = acts.tile([P, NT * D], F32, name="x_all", tag="x")
            dma(out=x_all.rearrange("p (j f) -> p j f", f=D),
                in_=x_in.rearrange("(j p) f -> p j f", p=P))

            def layernorm_z(z_all, gb, bb, x_out_name, x_out_tag, mask=None):
                zv = z_all.rearrange("p (j f) -> p j f", f=D)
                bn6 = stats.tile([P, NT, 6], F32, name=x_out_name + "_bn6",
                                 tag="bn6")
                mv = stats.tile([P, NT, 2], F32, name=x_out_name + "_mv",
                                tag="mv")
                for j in range(NT):
                    nc.vector.bn_stats(out=bn6[:, j, :], in_=zv[:, j, :])
                    nc.vector.bn_aggr(out=mv[:, j, :], in_=bn6[:, j, :])
                rstd = stats.tile([P, NT], F32, name=x_out_name + "_rstd",
                                  tag="rstd")
                vv = stats.tile([P, NT], F32, name=x_out_name + "_vv", tag="vv")
                nc.vector.tensor_scalar(out=vv, in0=mv[:, :, 1],
                                        scalar1=LN_EPS, scalar2=None,
                                        op0=Alu.add)
                iv = vv.bitcast(mybir.dt.int32)
                ir = rstd.bitcast(mybir.dt.int32)
                nc.vector.tensor_scalar(out=ir, in0=iv, scalar1=1,
                                        scalar2=None,
                                        op0=Alu.logical_shift_right)
                nc.vector.tensor_scalar(out=ir, in0=ir, scalar1=-1,
                                        scalar2=0x5F3759DF,
                                        op0=Alu.mult, op1=Alu.add)
                tq = stats.tile([P, NT], F32, name=x_out_name + "_tq", tag="tq")
                for _ in range(2):
                    nc.vector.tensor_mul(tq, rstd, rstd)
                    nc.vector.tensor_mul(tq, tq, vv)
                    nc.vector.tensor_scalar(out=tq, in0=tq, scalar1=-0.5,
                                            scalar2=1.5,
                                            op0=Alu.mult, op1=Alu.add)
                    nc.vector.tensor_mul(rstd, rstd, tq)
                o = acts.tile([P, NT * D], F32, name=x_out_name, tag=x_out_tag)
                for j in range(NT):
                    nc.vector.tensor_scalar(
                        out=o[:, j * D: (j + 1) * D],
                        in0=z_all[:, j * D: (j + 1) * D],
                        scalar1=mv[:, j, 0:1], scalar2=rstd[:, j: j + 1],
                        op0=Alu.subtract, op1=Alu.mult,
                    )
                nc.vector.tensor_mul(o, o, gb)
                nc.vector.tensor_add(o, o, bb)
                if mask is not None:
                    nc.vector.tensor_mul(o, o, mask)
                return o

            for l in range(NL):
                # per-layer LN/bias vectors broadcast: [128, 6, NT, D]
                # order: g1, be1, g2, be2, bo, b2
                lnv = lnp.tile([P, 6, NT, D], F32, name=f"lnv{l}", tag="lnv")
                for v in range(6):
                    lv = LNV[l, v]
                    dma(out=lnv[:, v], in_=bass.AP(
                        tensor=lv.tensor, offset=lv.offset,
                        ap=[[0, P], [0, NT], [1, D]]))
                g1b = lnv[:, 0].rearrange("p j f -> p (j f)")
                be1b = lnv[:, 1].rearrange("p j f -> p (j f)")
                g2b = lnv[:, 2].rearrange("p j f -> p (j f)")
                be2b = lnv[:, 3].rearrange("p j f -> p (j f)")
                bo_b = lnv[:, 4].rearrange("p j f -> p (j f)")
                b2_b = lnv[:, 5].rearrange("p j f -> p (j f)")

                # ======== xTaug (feature-major + ones row) ========
                pX = psA.tile([P, L], F32, name=f"pX{l}", tag="A")
                for j in range(NT):
                    nc.tensor.transpose(
                        out=pX[0:D, j * P: (j + 1) * P],
                        in_=x_all[:, j * D: (j + 1) * D], identity=I128,
                    )
                for half in range(2):
                    nc.vector.tensor_copy(
                        xTaug[0:D, half * 512: (half + 1) * 512],
                        pX[0:D, half * 512: (half + 1) * 512],
                    )

                # ======== QKV projections ========
                vta = []
                for g in range(NG):
                    pQ = psL.tile([P, L], F32, name=f"pQ{l}_{g}", tag="L")
                    for hf in range(2):
                        nc.tensor.matmul(pQ[:, hf * 512: (hf + 1) * 512],
                                         wqa_all[:, l, g, :],
                                         xTaug[:, hf * 512: (hf + 1) * 512])
                    nc.vector.tensor_tensor(qaT[g], pQ, keep_b128, op=Alu.mult)

                    pK = psL.tile([P, L], F32, name=f"pK{l}_{g}", tag="L")
                    for hf in range(2):
                        nc.tensor.matmul(pK[:, hf * 512: (hf + 1) * 512],
                                         wka_all[:, l, g, :],
                                         xTaug[:, hf * 512: (hf + 1) * 512])
                    nc.scalar.activation(out=kaT[g], in_=pK, func=Act.Copy)
                    for m in range(MPG):
                        nc.gpsimd.dma_start(
                            out=kaT[g][32 * m + 8: 32 * m + 9, :],
                            in_=padneg.rearrange("(o n) -> o n", o=1).bitcast(F32R))

                    pV = psL.tile([P, L], F32, name=f"pV{l}_{g}", tag="L")
                    for hf in range(2):
                        nc.tensor.matmul(pV[0:36, hf * 512: (hf + 1) * 512],
                                         wva_all[:, l, g, :],
                                         xTaug[:, hf * 512: (hf + 1) * 512])
                    vTx = acts.tile([36, L], F32, name=f"vTx{l}_{g}", tag="vTx")
                    nc.vector.tensor_tensor(vTx, pV[0:36, :], keep_b36, op=Alu.mult)
                    # token-major v-aug bf16 [128, NT, 36]
                    pVt = psL.tile([P, L], F32, name=f"pVt{l}_{g}", tag="L")
                    for j in range(NT):
                        nc.tensor.transpose(
                            out=pVt[:, j * P: j * P + 36],
                            in_=vTx[:, j * P: (j + 1) * P],
                            identity=I128[0:36, 0:36],
                        )
                    vt_g = acts.tile([P, NT, 36], BF16, name=f"vta{l}_{g}",
                                     tag=f"vta{g}")
                    nc.vector.tensor_copy(
                        vt_g, pVt.rearrange("p (j c) -> p j c", c=P)[:, :, 0:36])
                    vta.append(vt_g)

                # ======== attention ========
                mxs = stats.tile([P, H], F32, name=f"mxs{l}", tag="mxs")
                attnTa = []
                woaS_l = []
                sms = []
                for g in range(NG):
                    R_g = rpool.tile([P, MPG, L], BF16, name=f"R{l}_{g}", tag="R")
                    pAcc = psA.tile([P, L], F32, name=f"pAcc{l}_{g}", tag="A")
                    for j in range(NT):
                        et = epool.tile([P, MPG, L], BF16, name=f"e{l}_{g}_{j}",
                                        tag="e")
                        for kh in range(2):
                            for mp in range(2):
                                pL = psL.tile([P, L], F32,
                                              name=f"pL{l}_{g}_{j}_{kh}_{mp}",
                                              tag="L")
                                for mh in range(2):
                                    m = 2 * mp + mh
                                    rb = 32 * m
                                    nc.tensor.matmul(
                                        pL[:, mh * 512: (mh + 1) * 512],
                                        qaT[g][rb: rb + 9, j * P: (j + 1) * P],
                                        kaT[g][rb: rb + 9,
                                               kh * 512: (kh + 1) * 512],
                                        tile_position=(rb, 0),
                                    )
                                nc.scalar.activation(
                                    out=et[:, 2 * mp: 2 * mp + 2,
                                           kh * 512: (kh + 1) * 512],
                                    in_=pL.rearrange("p (m k) -> p m k", k=512),
                                    func=Act.Exp,
                                )
                        for kh in range(2):
                            for m in range(MPG):
                                nc.tensor.matmul(
                                    pAcc[32 * m: 32 * m + 9,
                                         kh * 512: (kh + 1) * 512],
                                    vta[g][:, j, 9 * m: 9 * m + 9],
                                    et[:, m, kh * 512: (kh + 1) * 512],
                                    start=(j == 0), stop=(j == NT - 1),
                                    skip_group_check=True,
                                    tile_position=(0, 32 * m),
                                )
                        if j == 0:
                            nc.vector.tensor_copy(R_g, et)
                        else:
                            nc.vector.tensor_tensor(R_g, R_g, et, op=Alu.max)
                    nc.vector.tensor_reduce(
                        out=mxs[:, MPG * g: MPG * (g + 1)], in_=R_g,
                        axis=mybir.AxisListType.X, op=Alu.max,
                    )
                    sm = stats.tile([P, 1], F32, name=f"sms{l}_{g}", tag=f"sms{g}")
                    nc.vector.reduce_sum(out=sm, in_=pAcc,
                                         axis=mybir.AxisListType.X)
                    sms.append(sm)
                    at = acts.tile([P, L], F32R, name=f"attnTa{l}_{g}",
                                   tag=f"at{g}")
                    nc.vector.tensor_copy(at, pAcc)
                    attnTa.append(at)

                    # nz / s_h broadcast over each 32-row block -> scale Wo rhs
                    pCp = psL.tile([P, L], F32, name=f"pCp{l}_{g}", tag="L")
                    nc.tensor.matmul(pCp[:, 0:1], b32t, sm)
                    rsc = stats.tile([P, 1], F32, name=f"rsc{l}_{g}",
                                     tag=f"rsc{g}")
                    nc.vector.reciprocal(out=rsc, in_=pCp[:, 0:1])
                    caf = stats.tile([P, 1], F32, name=f"caf{l}_{g}",
                                     tag=f"caf{g}")
                    nc.vector.tensor_mul(caf, nzc, rsc)
                    woaS = stats.tile([P, D], F32R, name=f"woaS{l}_{g}",
                                      tag=f"woaS{g}")
                    nc.vector.tensor_scalar(out=woaS, in0=woa_all[:, l, g, :],
                                            scalar1=caf, scalar2=None,
                                            op0=Alu.mult)
                    woaS_l.append(woaS)

                # ======== stats: t8[h] = nz*mx_h/s_h (column space) ========
                pS8 = psL.tile([P, L], F32, name=f"pS8{l}", tag="L")
                for g in range(NG):
                    nc.tensor.matmul(pS8[0:H, 0:1], selt[g], sms[g],
                                     start=(g == 0), stop=(g == NG - 1),
                                     skip_group_check=True)
                pS = psL.tile([P, L], F32, name=f"pS{l}", tag="L")
                nc.tensor.transpose(out=pS[0:H, 0:P], in_=mxs, identity=I128)
                mx8 = stats.tile([H, 1], F32, name=f"mx8{l}", tag="mx8")
                nc.vector.reduce_max(out=mx8, in_=pS[0:H, 0:P],
                                     axis=mybir.AxisListType.X)
                t1 = stats.tile([H, 1], F32, name=f"t1_{l}", tag="t1")
                nc.vector.tensor_mul(t1, mx8, nz8)
                rs8 = stats.tile([H, 1], F32, name=f"rs8_{l}", tag="rs8")
                nc.vector.reciprocal(out=rs8, in_=pS8[0:H, 0:1])
                t8 = stats.tile([H, 1], F32, name=f"t8_{l}", tag="t8")
                nc.vector.tensor_mul(t8, t1, rs8)

                # ======== AllReduce(max) of the per-head vector ========
                cc_in = dramp.tile([H, 1], F32, name=f"cc_in{l}", tag=f"cc_in{l}")
                cc_out = dramp.tile([H, 1], F32, name=f"cc_out{l}",
                                    tag=f"cc_out{l}", addr_space="Shared")
                nc.gpsimd.dma_start(out=cc_in[:], in_=t8)
                nc.gpsimd.collective_compute(
                    "AllReduce", Alu.max,
                    replica_groups=[list(range(NCORES))],
                    ins=[cc_in.opt()], outs=[cc_out.opt()],
                )
                Grow = stats.tile([1, H], F32, name=f"Grow{l}", tag="Grow")
                nc.gpsimd.dma_start(out=Grow, in_=cc_out[:].rearrange("a b -> b a"))
                G = stats.tile([1, 1], F32, name=f"G{l}", tag=f"G{l}")
                nc.vector.reduce_max(out=G, in_=Grow, axis=mybir.AxisListType.X)

                # xb = x + bo (independent of G; overlaps the collective)
                xb = acts.tile([P, NT * D], F32, name=f"xb{l}", tag="xb")
                nc.vector.tensor_add(xb, x_all, bo_b)

                # ======== Wo token-major (overlaps the AllReduce) ========
                pZ1 = psA.tile([P, L], F32, name=f"pZ1{l}", tag="A")
                for j in range(NT):
                    for g in range(NG):
                        nc.tensor.matmul(
                            pZ1[:, j * D: (j + 1) * D],
                            attnTa[g][:, j * P: (j + 1) * P],
                            woaS_l[g],
                            start=(g == 0), stop=(g == NG - 1),
                            skip_group_check=True,
                        )

                # 1/G broadcast [128,1]
                pRG = psL.tile([P, L], F32, name=f"pRG{l}", tag="L")
                nc.tensor.matmul(pRG[:, 0:1], ones_t[0:1, :], G)
                rGa = stats.tile([P, 1], F32, name=f"rGa{l}", tag="rGa")
                nc.vector.reciprocal(out=rGa, in_=pRG[:, 0:1])

                # z = attn/G + x + bo, then LN1
                z1 = acts.tile([P, NT * D], F32, name=f"z1_{l}", tag="z1")
                nc.vector.scalar_tensor_tensor(
                    out=z1, in0=pZ1[:, 0: NT * D], scalar=rGa,
                    in1=xb, op0=Alu.mult, op1=Alu.add,
                )
                out1_all = layernorm_z(z1, g1b, be1b, f"out1_{l}", "out1")

                # ======== o1Taug + FFN ========
                pO = psA.tile([P, L], F32, name=f"pO{l}", tag="A")
                for j in range(NT):
                    nc.tensor.transpose(
                        out=pO[0:D, j * P: (j + 1) * P],
                        in_=out1_all[:, j * D: (j + 1) * D], identity=I128,
                    )
                for half in range(2):
                    nc.vector.tensor_copy(
                        o1Taug[0:D, half * 512: (half + 1) * 512],
                        pO[0:D, half * 512: (half + 1) * 512],
                    )

                h1 = []
                for i in range(2):
                    pH = psL.tile([P, L], F32, name=f"pH{l}_{i}", tag="L")
                    for hf in range(2):
                        nc.tensor.matmul(pH[:, hf * 512: (hf + 1) * 512],
                                         w1a_all[:, l, i, :],
                                         o1Taug[:, hf * 512: (hf + 1) * 512])
                    h1x = acts.tile([P, L], F32R, name=f"h1_{l}_{i}",
                                    tag=f"h1_{i}")
                    nc.scalar.activation(out=h1x, in_=pH, func=Act.Relu)
                    h1.append(h1x)

                # xb2 = out1 + b2 (ready before W2 lands)
                xb2 = acts.tile([P, NT * D], F32, name=f"xb2{l}", tag="xb2")
                nc.vector.tensor_add(xb2, out1_all, b2_b)

                # W2 token-major
                pZ2 = psA.tile([P, L], F32, name=f"pZ2{l}", tag="A")
                for j in range(NT):
                    for i in range(2):
                        nc.tensor.matmul(
                            pZ2[:, j * D: (j + 1) * D],
                            h1[i][:, j * P: (j + 1) * P],
                            w2_all[:, l, i, :],
                            start=(i == 0), stop=(i == 1),
                            skip_group_check=True,
                        )
                z2 = acts.tile([P, NT * D], F32, name=f"z2_{l}", tag="z2")
                nc.vector.tensor_add(z2, pZ2[:, 0: NT * D], xb2)

                x_all = layernorm_z(z2, g2b, be2b, f"x_next_{l}", "x",
                                    mask=keep_exp2 if l == NL - 1 else None)

            dma(out=out.rearrange("(j p) f -> p j f", p=P),
                in_=x_all.rearrange("p (j f) -> p j f", f=D))

    return nc


_NC_CACHE = None


def _get_nc():
    global _NC_CACHE
    if _NC_CACHE is None:
        _NC_CACHE = build_bass()
    return _NC_CACHE


def _make_in_maps(inputs):
    x = np.asarray(inputs["x"], np.float32)
    protok = np.asarray(inputs["protok"])
    B = x.shape[0]
    keep = (protok != 0).astype(np.float32)
    padneg = (keep - 1.0) * NEG_BIG

    Wq = np.asarray(inputs["Wq"], np.float32)
    Wk = np.asarray(inputs["Wk"], np.float32)
    Wv = np.asarray(inputs["Wv"], np.float32)
    Wo = np.asarray(inputs["Wo"], np.float32)
    bq = np.asarray(inputs["bq"], np.float32)
    bk = np.asarray(inputs["bk"], np.float32)
    bv = np.asarray(inputs["bv"], np.float32)
    W1 = np.asarray(inputs["W1"], np.float32)
    b1 = np.asarray(inputs["b1"], np.float32)
    W2in = np.asarray(inputs["W2"], np.float32)

    WqA = np.zeros((NL, NG, D + 1, P), np.float32)
    WkA = np.zeros((NL, NG, D + 1, P), np.float32)
    WvA = np.zeros((NL, NG, D + 1, 36), np.float32)
    WoA = np.zeros((NL, NG, P, D), np.float32)
    for g in range(NG):
        for m in range(MPG):
            h = MPG * g + m
            cols = slice(DH * h, DH * (h + 1))
            WqA[:, g, 0:D, 32 * m: 32 * m + DH] = Wq[:, :, cols]
            WqA[:, g, D, 32 * m: 32 * m + DH] = bq[:, cols]
            WqA[:, g, D, 32 * m + 8] = 1.0
            WkA[:, g, 0:D, 32 * m: 32 * m + DH] = Wk[:, :, cols]
            WkA[:, g, D, 32 * m: 32 * m + DH] = bk[:, cols]
            WvA[:, g, 0:D, 9 * m: 9 * m + DH] = Wv[:, :, cols]
            WvA[:, g, D, 9 * m: 9 * m + DH] = bv[:, cols]
            WvA[:, g, D, 9 * m + 8] = 1.0
            WoA[:, g, 32 * m: 32 * m + DH, :] = Wo[:, cols, :]

    W1A = np.zeros((NL, 2, D + 1, P), np.float32)
    for i in range(2):
        W1A[:, i, 0:D, :] = W1[:, :, i * P: (i + 1) * P]
        W1A[:, i, D, :] = b1[:, i * P: (i + 1) * P]
    W2A = np.ascontiguousarray(W2in.reshape(NL, 2, P, D))

    LNVc = np.stack([
        np.asarray(inputs["g1"], np.float32),
        np.asarray(inputs["be1"], np.float32),
        np.asarray(inputs["g2"], np.float32),
        np.asarray(inputs["be2"], np.float32),
        np.asarray(inputs["bo"], np.float32),
        np.asarray(inputs["b2"], np.float32),
    ], axis=1)  # [NL, 6, D]

    B32c = np.zeros((P, P), np.float32)
    for r in range(P):
        B32c[32 * (r // 32) + 8, r] = 1.0
    SELSc = np.zeros((NG, P, H), np.float32)
    for g in range(NG):
        for m in range(MPG):
            SELSc[g, 32 * m + 8, MPG * g + m] = 1.0

    shared = dict(
        WqA=WqA, WkA=WkA, WvA=WvA, WoA=WoA,
        W1A=W1A, W2=W2A, LNV=np.ascontiguousarray(LNVc),
        B32=B32c, SELS=SELSc,
    )
    in_maps = []
    for i in range(NCORES):
        b = i % B
        in_maps.append(dict(
            x=np.ascontiguousarray(x[b]),
            keepf=np.ascontiguousarray(keep[b]),
            padneg=np.ascontiguousarray(padneg[b]),
            **shared,
        ))
    return in_maps


def run_on_hw(inputs, trace=False, **kwargs):
    from concourse.bass_utils import run_bass_kernel_spmd

    nc = _get_nc()
    if not nc.is_finalized():
        nc.finalize()
    in_maps = _make_in_maps(inputs)
    res = run_bass_kernel_spmd(nc, in_maps, list(range(NCORES)), trace=trace, **kwargs)
    outs = np.stack([res.results[i]["out"] for i in range(NCORES)], axis=0)
    return outs.astype(np.float32), res


def kernel(**inputs):
    outs, _ = run_on_hw(inputs, trace=False)
    return outs


# revision 15
# speedup vs baseline: 1.3846x; 1.0041x over previous
"""Bass/Trainium2 kernel for nn_Encoders_6751688590031.

4-layer transformer encoder, d_model=64, H=8 heads, L=1024, dff=256, B=8.
Sharding: data-parallel over batch across 8 NeuronCores (1 batch element
per core); weights replicated. One AllReduce(max) per layer for the global
jnp.max(w) softmax normalization.

v3 design:
 - Logits: 4 heads packed per PE pass via 32-row tiling (K=9 row-tiles
   run concurrently). qaT/kaT hold 4 heads per group at 32-row pitch;
   row 32m+8 = ones (q) / padneg (k).
 - e^T v: v-stationary col-tiled matmuls (M=9 incl. ones column that
   accumulates per-head colsums -> s_h), PSUM-accumulated over q tiles.
   Output is an augmented feature-major attnT [128, L].
 - Wo and W2 applied TOKEN-major (lhsT = activation tile, rhs = weight),
   so no transposes / evacuations before the LNs; residual+bias adds are
   fused TTs from PSUM.
 - nz/s_h is folded into Wo's rhs before the AllReduce; the global 1/G
   is a post-AllReduce scalar fused into the residual add, so the Wo
   matmuls overlap the collective.
 - Per-head max of e: running bf16 TT-max folds (2x DVE) + final gpsimd
   XYZWC reduces (off the DVE critical path).
 - exp(): [128,1024] PSUM->SBUF bf16, strided out, 3-deep PSUM rotation.
 - Biases via augmented ones rows; grouped bn_stats; 2-step Newton rsqrt.
"""

import os
import sys

import numpy as np

for _p in (
    "/root/.axon_site",
    "/root/.axon_site/_ro/trn_rl_repo",
    "/root/.axon_site/_ro/pypackages",
    "/opt/trn_rl_repo",
):
    if os.path.isdir(_p) and _p not in sys.path:
        sys.path.append(_p)

import concourse.bass as bass
import concourse.bacc as bacc
import concourse.tile as tile
from concourse import mybir

F32 = mybir.dt.float32
F32R = mybir.dt.float32r
BF16 = mybir.dt.bfloat16

L = 1024          # sequence length
D = 64            # d_model
H = 8             # heads
DH = 8            # head dim
DFF = 256
NL = 4            # layers
P = 128           # partitions per token tile
NT = L // P       # 8 token tiles
NG = 2            # head groups (4 heads each)
MPG = 4           # heads per group
NCORES = 8
NEG_BIG = 1.0e9
LN_EPS = 1e-9

Act = mybir.ActivationFunctionType
Alu = mybir.AluOpType


def _r(ap):
    return ap.bitcast(F32R)


def build_bass():
    nc = bacc.Bacc(
        "TRN2", target_bir_lowering=False, debug=False, num_devices=NCORES
    )

    x_in = nc.declare_dram_parameter("x", [L, D], F32, isOutput=False)
    keepf = nc.declare_dram_parameter("keepf", [L], F32, isOutput=False)
    padneg = nc.declare_dram_parameter("padneg", [L], F32, isOutput=False)
    WqA = nc.declare_dram_parameter("WqA", [NL, NG, D + 1, P], F32, isOutput=False)
    WkA = nc.declare_dram_parameter("WkA", [NL, NG, D + 1, P], F32, isOutput=False)
    WvA = nc.declare_dram_parameter("WvA", [NL, NG, D + 1, 36], F32, isOutput=False)
    WoA = nc.declare_dram_parameter("WoA", [NL, NG, P, D], F32, isOutput=False)
    W1A = nc.declare_dram_parameter("W1A", [NL, 2, D + 1, P], F32, isOutput=False)
    W2 = nc.declare_dram_parameter("W2", [NL, 2, P, D], BF16, isOutput=False)
    LNV = nc.declare_dram_parameter("LNV", [NL, 6, D], F32, isOutput=False)
    B32 = nc.declare_dram_parameter("B32", [P, P], F32, isOutput=False)
    SELS = nc.declare_dram_parameter("SELS", [NG, P, MPG], F32, isOutput=False)
    out = nc.declare_dram_parameter("out", [L, D], F32, isOutput=True)

    dma = nc.sync.dma_start

    with tile.TileContext(nc) as tc:
        with (
            tc.tile_pool(name="const", bufs=1) as constp,
            tc.tile_pool(name="wpool", bufs=1) as wp,
            tc.tile_pool(name="lnp", bufs=2) as lnp,
            tc.tile_pool(name="persist", bufs=1) as pers,
            tc.tile_pool(name="acts", bufs=2) as acts,
            tc.tile_pool(name="epool", bufs=3) as epool,
            tc.tile_pool(name="rpool", bufs=2) as rpool,
            tc.tile_pool(name="stats", bufs=2) as stats,
            tc.tile_pool(name="psL", bufs=3, space="PSUM") as psL,
            tc.tile_pool(name="psA", bufs=1, space="PSUM") as psA,
            tc.tile_pool(name="dram", bufs=1, space="DRAM") as dramp,
        ):
            # ================= constants =================
            ones_t = constp.tile([P, P], F32, name="ones_t")
            nc.vector.memset(ones_t, 1.0)
            I128 = constp.tile([P, P], F32, name="I128")
            nc.gpsimd.affine_select(
                out=I128, in_=ones_t, pattern=[[-1, P]],
                compare_op=Alu.is_equal, fill=0.0, base=0, channel_multiplier=1,
            )
            ones_row8 = constp.tile([1, H], F32, name="ones_row8")
            nc.vector.memset(ones_row8, 1.0)
            ones_roww = constp.tile([1, L], F32, name="ones_roww")
            nc.vector.memset(ones_roww, 1.0)

            # zero psA banks once: never-written quadrant rows of the e^T v
            # accumulator then stay finite forever after
            pZI = psA.tile([P, L], F32, name="pZI", tag="A")
            nc.vector.memset(pZI, 0.0)

            def bcast_ap(dram_vec, parts, inner):
                return bass.AP(
                    tensor=dram_vec.tensor, offset=dram_vec.offset,
                    ap=[[0, parts]] + dram_vec.ap,
                )

            # keep broadcast [128, L]; rows 32m+8 stay 1.0 (ones rows)
            keep_b128 = constp.tile([P, L], F32, name="keep_b128")
            segs = [(0, 8), (9, 40), (41, 72), (73, 104), (105, 128)]
            for a, b in segs:
                dma(out=keep_b128[a:b, :], in_=bcast_ap(keepf[:], b - a, L))
            for m in range(MPG):
                nc.gpsimd.dma_start(
                    out=keep_b128[32 * m + 8: 32 * m + 9, :], in_=ones_roww[:])
            # keep broadcast [36, L] for v-aug; rows 9m+8 stay 1.0
            keep_b36 = constp.tile([36, L], F32, name="keep_b36")
            for m in range(MPG):
                dma(out=keep_b36[9 * m: 9 * m + 8, :], in_=bcast_ap(keepf[:], 8, L))
                nc.gpsimd.dma_start(
                    out=keep_b36[9 * m + 8: 9 * m + 9, :], in_=ones_roww[:])

            # token-major keep for nz and the final output mask
            keep_JP = constp.tile([NT, P], F32, name="keep_JP")
            dma(out=keep_JP, in_=keepf.rearrange("(j p) -> j p", p=P))
            pKA = psL.tile([P, L], F32, name="pKA", tag="L")
            nc.tensor.transpose(out=pKA[:, 0:NT], in_=keep_JP,
                                identity=I128[0:NT, 0:NT])
            keep_all = constp.tile([P, NT], F32, name="keep_all")
            nc.vector.tensor_copy(keep_all, pKA[:, 0:NT])
            keep_exp = constp.tile([P, NT, D], F32, name="keep_exp")
            for j in range(NT):
                nc.vector.tensor_scalar(
                    out=keep_exp[:, j, :], in0=ones_t[:, 0:D],
                    scalar1=keep_all[:, j: j + 1], scalar2=None, op0=Alu.mult,
                )
            keep_exp2 = keep_exp.rearrange("p j f -> p (j f)")

            # nz scalars: nz1 [1,1], nz8row [1,8], nzc [128,1]
            nzk = constp.tile([P, 1], F32, name="nzk")
            nc.vector.reduce_sum(out=nzk, in_=keep_all, axis=mybir.AxisListType.X)
            pNZ = psL.tile([P, L], F32, name="pNZ", tag="L")
            nc.tensor.matmul(pNZ[0:1, 0:1], ones_t[:, 0:1], nzk)
            nz1 = constp.tile([1, 1], F32, name="nz1")
            nc.vector.tensor_copy(nz1, pNZ[0:1, 0:1])
            pNZ8 = psL.tile([P, L], F32, name="pNZ8", tag="L")
            nc.tensor.matmul(pNZ8[0:H, 0:1], ones_row8, nz1)
            nc.tensor.matmul(pNZ8[:, 2:3], ones_t[0:1, :], nz1)
            nz8 = constp.tile([H, 1], F32, name="nz8")
            nc.vector.tensor_copy(nz8, pNZ8[0:H, 0:1])
            nzc = constp.tile([P, 1], F32, name="nzc")
            nc.vector.tensor_copy(nzc, pNZ8[:, 2:3])

            b32t = constp.tile([P, P], F32, name="b32t")
            dma(out=b32t, in_=B32[:, :])
            selt = []
            for g in range(NG):
                t2 = constp.tile([P, MPG], F32, name=f"selt{g}")
                dma(out=t2, in_=SELS[g])
                selt.append(t2)

            # ================= weights (consolidated DMAs) =================
            wqa_all = wp.tile([D + 1, NL, NG, P], F32R, name="wqa_all")
            dma(out=wqa_all, in_=_r(WqA[:, :, :, :].rearrange("l g r c -> r l g c")))
            wka_all = wp.tile([D + 1, NL, NG, P], F32R, name="wka_all")
            dma(out=wka_all, in_=_r(WkA[:, :, :, :].rearrange("l g r c -> r l g c")))
            wva_all = wp.tile([D + 1, NL, NG, 36], F32R, name="wva_all")
            dma(out=wva_all, in_=_r(WvA[:, :, :, :].rearrange("l g r c -> r l g c")))
            woa_all = wp.tile([P, NL, NG, D], F32, name="woa_all")
            dma(out=woa_all, in_=WoA[:, :, :, :].rearrange("l g r c -> r l g c"))
            w1a_all = wp.tile([D + 1, NL, 2, P], F32R, name="w1a_all")
            dma(out=w1a_all, in_=_r(W1A[:, :, :, :].rearrange("l i r c -> r l i c")))
            w2_all = wp.tile([P, NL, 2, D], BF16, name="w2_all")
            dma(out=w2_all, in_=W2[:, :, :, :].rearrange("l i r c -> r l i c"))

            # persistent augmented feature-major activations
            xTaug = pers.tile([D + 1, L], F32R, name="xTaug")
            nc.gpsimd.dma_start(out=xTaug[D: D + 1, :],
                                in_=ones_roww[:].bitcast(F32R))
            o1Taug = pers.tile([D + 1, L], F32R, name="o1Taug")
            nc.gpsimd.dma_start(out=o1Taug[D: D + 1, :],
                                in_=ones_roww[:].bitcast(F32R))
            qaT = [pers.tile([P, L], F32R, name=f"qaT{g}") for g in range(NG)]
            kaT = [pers.tile([P, L], F32R, name=f"kaT{g}") for g in range(NG)]

            # ---- initial x load: token-major packed [128, (j,f)] ----
            x_all = acts.tile([P, NT * D], F32, name="x_all", tag="x")
            dma(out=x_all.rearrange("p (j f) -> p j f", f=D),
                in_=x_in.rearrange("(j p) f -> p j f", p=P))

            def layernorm_z(z_all, gb, bb, x_out_name, x_out_tag, mask=None):
                zv = z_all.rearrange("p (j f) -> p j f", f=D)
                bn6 = stats.tile([P, NT, 6], F32, name=x_out_name + "_bn6",
                                 tag="bn6")
                mv = stats.tile([P, NT, 2], F32, name=x_out_name + "_mv",
                                tag="mv")
                for j in range(NT):
                    nc.vector.bn_stats(out=bn6[:, j, :], in_=zv[:, j, :])
                    nc.vector.bn_aggr(out=mv[:, j, :], in_=bn6[:, j, :])
                rstd = stats.tile([P, NT], F32, name=x_out_name + "_rstd",
                                  tag="rstd")
                vv = stats.tile([P, NT], F32, name=x_out_name + "_vv", tag="vv")
                nc.vector.tensor_scalar(out=vv, in0=mv[:, :, 1],
                                        scalar1=LN_EPS, scalar2=None,
                                        op0=Alu.add)
                iv = vv.bitcast(mybir.dt.int32)
                ir = rstd.bitcast(mybir.dt.int32)
                nc.vector.tensor_scalar(out=ir, in0=iv, scalar1=1,
                                        scalar2=None,
                                        op0=Alu.logical_shift_right)
                nc.vector.tensor_scalar(out=ir, in0=ir, scalar1=-1,
                                        scalar2=0x5F3759DF,
                                        op0=Alu.mult, op1=Alu.add)
                tq = stats.tile([P, NT], F32, name=x_out_name + "_tq", tag="tq")
                for _ in range(1):
                    nc.vector.tensor_mul(tq, rstd, rstd)
                    nc.vector.tensor_mul(tq, tq, vv)
                    nc.vector.tensor_scalar(out=tq, in0=tq, scalar1=-0.5,
                                            scalar2=1.5,
                                            op0=Alu.mult, op1=Alu.add)
                    nc.vector.tensor_mul(rstd, rstd, tq)
                o = acts.tile([P, NT * D], F32, name=x_out_name, tag=x_out_tag)
                for j in range(NT):
                    nc.vector.tensor_scalar(
                        out=o[:, j * D: (j + 1) * D],
                        in0=z_all[:, j * D: (j + 1) * D],
                        scalar1=mv[:, j, 0:1], scalar2=rstd[:, j: j + 1],
                        op0=Alu.subtract, op1=Alu.mult,
                    )
                nc.vector.tensor_mul(o, o, gb)
                nc.vector.tensor_add(o, o, bb)
                if mask is not None:
                    nc.vector.tensor_mul(o, o, mask)
                return o

            for l in range(NL):
                # per-layer LN/bias vectors broadcast: [128, 6, NT, D]
                # order: g1, be1, g2, be2, bo, b2
                lnv = lnp.tile([P, 6, NT, D], F32, name=f"lnv{l}", tag="lnv")
                for v in range(6):
                    lv = LNV[l, v]
                    dma(out=lnv[:, v], in_=bass.AP(
                        tensor=lv.tensor, offset=lv.offset,
                        ap=[[0, P], [0, NT], [1, D]]))
                g1b = lnv[:, 0].rearrange("p j f -> p (j f)")
                be1b = lnv[:, 1].rearrange("p j f -> p (j f)")
                g2b = lnv[:, 2].rearrange("p j f -> p (j f)")
                be2b = lnv[:, 3].rearrange("p j f -> p (j f)")
                bo_b = lnv[:, 4].rearrange("p j f -> p (j f)")
                b2_b = lnv[:, 5].rearrange("p j f -> p (j f)")

                # ======== xTaug (feature-major + ones row) ========
                pX = psA.tile([P, L], F32, name=f"pX{l}", tag="A")
                for j in range(NT):
                    nc.tensor.transpose(
                        out=pX[0:D, j * P: (j + 1) * P],
                        in_=x_all[:, j * D: (j + 1) * D], identity=I128,
                    )
                for half in range(2):
                    nc.vector.tensor_copy(
                        xTaug[0:D, half * 512: (half + 1) * 512],
                        pX[0:D, half * 512: (half + 1) * 512],
                    )

                # ======== QKV projections ========
                vta = []
                for g in range(NG):
                    pQ = psL.tile([P, L], F32, name=f"pQ{l}_{g}", tag="L")
                    for hf in range(2):
                        nc.tensor.matmul(pQ[:, hf * 512: (hf + 1) * 512],
                                         wqa_all[:, l, g, :],
                                         xTaug[:, hf * 512: (hf + 1) * 512])
                    nc.vector.tensor_tensor(qaT[g], pQ, keep_b128, op=Alu.mult)

                    pK = psL.tile([P, L], F32, name=f"pK{l}_{g}", tag="L")
                    for hf in range(2):
                        nc.tensor.matmul(pK[:, hf * 512: (hf + 1) * 512],
                                         wka_all[:, l, g, :],
                                         xTaug[:, hf * 512: (hf + 1) * 512])
                    nc.scalar.activation(out=kaT[g], in_=pK, func=Act.Copy)
                    for m in range(MPG):
                        nc.gpsimd.dma_start(
                            out=kaT[g][32 * m + 8: 32 * m + 9, :],
                            in_=padneg.rearrange("(o n) -> o n", o=1).bitcast(F32R))

                    pV = psL.tile([P, L], F32, name=f"pV{l}_{g}", tag="L")
                    for hf in range(2):
                        nc.tensor.matmul(pV[0:36, hf * 512: (hf + 1) * 512],
                                         wva_all[:, l, g, :],
                                         xTaug[:, hf * 512: (hf + 1) * 512])
                    vTx = acts.tile([36, L], F32, name=f"vTx{l}_{g}", tag="vTx")
                    nc.vector.tensor_tensor(vTx, pV[0:36, :], keep_b36, op=Alu.mult)
                    # token-major v-aug bf16 [128, NT, 36]
                    pVt = psL.tile([P, L], F32, name=f"pVt{l}_{g}", tag="L")
                    for j in range(NT):
                        nc.tensor.transpose(
                            out=pVt[:, j * P: j * P + 36],
                            in_=vTx[:, j * P: (j + 1) * P],
                            identity=I128[0:36, 0:36],
                        )
                    vt_g = acts.tile([P, NT, 36], BF16, name=f"vta{l}_{g}",
                                     tag=f"vta{g}")
                    nc.vector.tensor_copy(
                        vt_g, pVt.rearrange("p (j c) -> p j c", c=P)[:, :, 0:36])
                    vta.append(vt_g)

                # ======== attention ========
                attnTa = []
                woaS_l = []
                cc_outs = []
                for g in range(NG):
                    R_g = rpool.tile([P, MPG, L], BF16, name=f"R{l}_{g}", tag="R")
                    pAcc = psA.tile([P, L], F32, name=f"pAcc{l}_{g}", tag="A")
                    for j in range(NT):
                        et = epool.tile([P, MPG, L], BF16, name=f"e{l}_{g}_{j}",
                                        tag="e")
                        for kh in range(2):
                            for mp in range(2):
                                pL = psL.tile([P, L], F32,
                                              name=f"pL{l}_{g}_{j}_{kh}_{mp}",
                                              tag="L")
                                for mh in range(2):
                                    m = 2 * mp + mh
                                    rb = 32 * m
                                    nc.tensor.matmul(
                                        pL[:, mh * 512: (mh + 1) * 512],
                                        qaT[g][rb: rb + 9, j * P: (j + 1) * P],
                                        kaT[g][rb: rb + 9,
                                               kh * 512: (kh + 1) * 512],
                                        tile_position=(rb, 0),
                                    )
                                nc.scalar.activation(
                                    out=et[:, 2 * mp: 2 * mp + 2,
                                           kh * 512: (kh + 1) * 512],
                                    in_=pL.rearrange("p (m k) -> p m k", k=512),
                                    func=Act.Exp,
                                )
                        for kh in range(2):
                            for m in range(MPG):
                                nc.tensor.matmul(
                                    pAcc[32 * m: 32 * m + 9,
                                         kh * 512: (kh + 1) * 512],
                                    vta[g][:, j, 9 * m: 9 * m + 9],
                                    et[:, m, kh * 512: (kh + 1) * 512],
                                    start=(j == 0), stop=(j == NT - 1),
                                    skip_group_check=True,
                                    tile_position=(0, 32 * m),
                                )
                        if j == 0:
                            nc.vector.tensor_copy(R_g, et)
                        else:
                            nc.vector.tensor_tensor(R_g, R_g, et, op=Alu.max)
                    sm = stats.tile([P, 1], F32, name=f"sms{l}_{g}", tag=f"sms{g}")
                    nc.vector.reduce_sum(out=sm, in_=pAcc,
                                         axis=mybir.AxisListType.X)
                    at = acts.tile([P, L], BF16, name=f"attnTa{l}_{g}",
                                   tag=f"at{g}")
                    nc.vector.tensor_copy(at, pAcc)
                    attnTa.append(at)

                    # nz / s_h broadcast over each 32-row block -> scale Wo rhs
                    pCp = psL.tile([P, L], F32, name=f"pCp{l}_{g}", tag="L")
                    nc.tensor.matmul(pCp[:, 0:1], b32t, sm)
                    nc.tensor.matmul(pCp[0:MPG, 1:2], selt[g], sm)
                    rsc = stats.tile([P, 1], F32, name=f"rsc{l}_{g}",
                                     tag=f"rsc{g}")
                    nc.vector.reciprocal(out=rsc, in_=pCp[:, 0:1])
                    caf = stats.tile([P, 1], F32, name=f"caf{l}_{g}",
                                     tag=f"caf{g}")
                    nc.vector.tensor_mul(caf, nzc, rsc)
                    woaS = stats.tile([P, D], BF16, name=f"woaS{l}_{g}",
                                      tag=f"woaS{g}")
                    nc.vector.tensor_scalar(out=woaS, in0=woa_all[:, l, g, :],
                                            scalar1=caf, scalar2=None,
                                            op0=Alu.mult)
                    woaS_l.append(woaS)

                    # per-grp t4 = nz*mx/s and its own AllReduce; grp0's
                    # collective is fully hidden under grp1's attention
                    mxs = stats.tile([P, MPG], F32, name=f"mxs{l}_{g}",
                                     tag=f"mxs{g}")
                    nc.vector.tensor_reduce(
                        out=mxs, in_=R_g, axis=mybir.AxisListType.X, op=Alu.max)
                    pS = psL.tile([P, L], F32, name=f"pS{l}_{g}", tag="L")
                    nc.tensor.transpose(out=pS[0:MPG, 0:P], in_=mxs,
                                        identity=I128)
                    mx4 = stats.tile([MPG, 1], F32, name=f"mx4{l}_{g}",
                                     tag=f"mx4{g}")
                    nc.vector.reduce_max(out=mx4, in_=pS[0:MPG, 0:P],
                                         axis=mybir.AxisListType.X)
                    t1 = stats.tile([MPG, 1], F32, name=f"t1_{l}_{g}",
                                    tag=f"t1{g}")
                    nc.vector.tensor_mul(t1, mx4, nz8[0:MPG, :])
                    rs4 = stats.tile([MPG, 1], F32, name=f"rs4_{l}_{g}",
                                     tag=f"rs4{g}")
                    nc.vector.reciprocal(out=rs4, in_=pCp[0:MPG, 1:2])
                    t4 = stats.tile([MPG, 1], F32, name=f"t4_{l}_{g}",
                                    tag=f"t4{g}")
                    nc.vector.tensor_mul(t4, t1, rs4)
                    cc_in = dramp.tile([MPG, 1], F32, name=f"cc_in{l}_{g}",
                                       tag=f"cc_in{l}_{g}")
                    cc_out = dramp.tile([MPG, 1], F32, name=f"cc_out{l}_{g}",
                                        tag=f"cc_out{l}_{g}",
                                        addr_space="Shared")
                    nc.gpsimd.dma_start(out=cc_in[:], in_=t4)
                    nc.gpsimd.collective_compute(
                        "AllReduce", Alu.max,
                        replica_groups=[list(range(NCORES))],
                        ins=[cc_in.opt()], outs=[cc_out.opt()],
                    )
                    cc_outs.append(cc_out)

                Grow = stats.tile([1, H], F32, name=f"Grow{l}", tag="Grow")
                for g in range(NG):
                    nc.gpsimd.dma_start(
                        out=Grow[:, MPG * g: MPG * (g + 1)],
                        in_=cc_outs[g][:].rearrange("a b -> b a"))
                G = stats.tile([1, 1], F32, name=f"G{l}", tag=f"G{l}")
                nc.vector.reduce_max(out=G, in_=Grow, axis=mybir.AxisListType.X)

                # xb = x + bo (independent of G; overlaps the collective)
                xb = acts.tile([P, NT * D], F32, name=f"xb{l}", tag="xb")
                nc.vector.tensor_add(xb, x_all, bo_b)

                # ======== Wo token-major (overlaps the AllReduce) ========
                pZ1 = psA.tile([P, L], F32, name=f"pZ1{l}", tag="A")
                for j in range(NT):
                    for g in range(NG):
                        nc.tensor.matmul(
                            pZ1[:, j * D: (j + 1) * D],
                            attnTa[g][:, j * P: (j + 1) * P],
                            woaS_l[g],
                            start=(g == 0), stop=(g == NG - 1),
                            skip_group_check=True,
                        )

                # 1/G broadcast [128,1]
                pRG = psL.tile([P, L], F32, name=f"pRG{l}", tag="L")
                nc.tensor.matmul(pRG[:, 0:1], ones_t[0:1, :], G)
                rGa = stats.tile([P, 1], F32, name=f"rGa{l}", tag="rGa")
                nc.vector.reciprocal(out=rGa, in_=pRG[:, 0:1])

                # z = attn/G + x + bo, then LN1
                z1 = acts.tile([P, NT * D], F32, name=f"z1_{l}", tag="z1")
                nc.vector.scalar_tensor_tensor(
                    out=z1, in0=pZ1[:, 0: NT * D], scalar=rGa,
                    in1=xb, op0=Alu.mult, op1=Alu.add,
                )
                out1_all = layernorm_z(z1, g1b, be1b, f"out1_{l}", "out1")

                # ======== o1Taug + FFN ========
                pO = psA.tile([P, L], F32, name=f"pO{l}", tag="A")
                for j in range(NT):
                    nc.tensor.transpose(
                        out=pO[0:D, j * P: (j + 1) * P],
                        in_=out1_all[:, j * D: (j + 1) * D], identity=I128,
                    )
                for half in range(2):
                    nc.vector.tensor_copy(
                        o1Taug[0:D, half * 512: (half + 1) * 512],
                        pO[0:D, half * 512: (half + 1) * 512],
                    )

                h1 = []
                for i in range(2):
                    pH = psL.tile([P, L], F32, name=f"pH{l}_{i}", tag="L")
                    for hf in range(2):
                        nc.tensor.matmul(pH[:, hf * 512: (hf + 1) * 512],
                                         w1a_all[:, l, i, :],
                                         o1Taug[:, hf * 512: (hf + 1) * 512])
                    h1x = acts.tile([P, L], BF16, name=f"h1_{l}_{i}",
                                    tag=f"h1_{i}")
                    nc.scalar.activation(out=h1x, in_=pH, func=Act.Relu)
                    h1.append(h1x)

                # xb2 = out1 + b2 (ready before W2 lands)
                xb2 = acts.tile([P, NT * D], F32, name=f"xb2{l}", tag="xb2")
                nc.vector.tensor_add(xb2, out1_all, b2_b)

                # W2 token-major
                pZ2 = psA.tile([P, L], F32, name=f"pZ2{l}", tag="A")
                for j in range(NT):
                    for i in range(2):
                        nc.tensor.matmul(
                            pZ2[:, j * D: (j + 1) * D],
                            h1[i][:, j * P: (j + 1) * P],
                            w2_all[:, l, i, :],
                            start=(i == 0), stop=(i == 1),
                            skip_group_check=True,
                        )
                z2 = acts.tile([P, NT * D], F32, name=f"z2_{l}", tag="z2")
                nc.vector.tensor_add(z2, pZ2[:, 0: NT * D], xb2)

                x_all = layernorm_z(z2, g2b, be2b, f"x_next_{l}", "x",
                                    mask=keep_exp2 if l == NL - 1 else None)

            dma(out=out.rearrange("(j p) f -> p j f", p=P),
                in_=x_all.rearrange("p (j f) -> p j f", f=D))

    return nc


_NC_CACHE = None


def _get_nc():
    global _NC_CACHE
    if _NC_CACHE is None:
        _NC_CACHE = build_bass()
    return _NC_CACHE


def _make_in_maps(inputs):
    x = np.asarray(inputs["x"], np.float32)
    protok = np.asarray(inputs["protok"])
    B = x.shape[0]
    keep = (protok != 0).astype(np.float32)
    padneg = (keep - 1.0) * NEG_BIG

    Wq = np.asarray(inputs["Wq"], np.float32)
    Wk = np.asarray(inputs["Wk"], np.float32)
    Wv = np.asarray(inputs["Wv"], np.float32)
    Wo = np.asarray(inputs["Wo"], np.float32)
    bq = np.asarray(inputs["bq"], np.float32)
    bk = np.asarray(inputs["bk"], np.float32)
    bv = np.asarray(inputs["bv"], np.float32)
    W1 = np.asarray(inputs["W1"], np.float32)
    b1 = np.asarray(inputs["b1"], np.float32)
    W2in = np.asarray(inputs["W2"], np.float32)

    WqA = np.zeros((NL, NG, D + 1, P), np.float32)
    WkA = np.zeros((NL, NG, D + 1, P), np.float32)
    WvA = np.zeros((NL, NG, D + 1, 36), np.float32)
    WoA = np.zeros((NL, NG, P, D), np.float32)
    for g in range(NG):
        for m in range(MPG):
            h = MPG * g + m
            cols = slice(DH * h, DH * (h + 1))
            WqA[:, g, 0:D, 32 * m: 32 * m + DH] = Wq[:, :, cols]
            WqA[:, g, D, 32 * m: 32 * m + DH] = bq[:, cols]
            WqA[:, g, D, 32 * m + 8] = 1.0
            WkA[:, g, 0:D, 32 * m: 32 * m + DH] = Wk[:, :, cols]
            WkA[:, g, D, 32 * m: 32 * m + DH] = bk[:, cols]
            WvA[:, g, 0:D, 9 * m: 9 * m + DH] = Wv[:, :, cols]
            WvA[:, g, D, 9 * m: 9 * m + DH] = bv[:, cols]
            WvA[:, g, D, 9 * m + 8] = 1.0
            WoA[:, g, 32 * m: 32 * m + DH, :] = Wo[:, cols, :]

    W1A = np.zeros((NL, 2, D + 1, P), np.float32)
    for i in range(2):
        W1A[:, i, 0:D, :] = W1[:, :, i * P: (i + 1) * P]
        W1A[:, i, D, :] = b1[:, i * P: (i + 1) * P]
    import ml_dtypes
    W2A = np.ascontiguousarray(
        W2in.reshape(NL, 2, P, D).astype(ml_dtypes.bfloat16))

    LNVc = np.stack([
        np.asarray(inputs["g1"], np.float32),
        np.asarray(inputs["be1"], np.float32),
        np.asarray(inputs["g2"], np.float32),
        np.asarray(inputs["be2"], np.float32),
        np.asarray(inputs["bo"], np.float32),
        np.asarray(inputs["b2"], np.float32),
    ], axis=1)  # [NL, 6, D]

    B32c = np.zeros((P, P), np.float32)
    for r in range(P):
        B32c[32 * (r // 32) + 8, r] = 1.0
    SELSc = np.zeros((NG, P, MPG), np.float32)
    for g in range(NG):
        for m in range(MPG):
            SELSc[g, 32 * m + 8, m] = 1.0

    shared = dict(
        WqA=WqA, WkA=WkA, WvA=WvA, WoA=WoA,
        W1A=W1A, W2=W2A, LNV=np.ascontiguousarray(LNVc),
        B32=B32c, SELS=SELSc,
    )
    in_maps = []
    for i in range(NCORES):
        b = i % B
        in_maps.append(dict(
            x=np.ascontiguousarray(x[b]),
            keepf=np.ascontiguousarray(keep[b]),
            padneg=np.ascontiguousarray(padneg[b]),
            **shared,
        ))
    return in_maps


def run_on_hw(inputs, trace=False, **kwargs):
    from concourse.bass_utils import run_bass_kernel_spmd

    nc = _get_nc()
    if not nc.is_finalized():
        nc.finalize()
    in_maps = _make_in_maps(inputs)
    res = run_bass_kernel_spmd(nc, in_maps, list(range(NCORES)), trace=trace, **kwargs)
    outs = np.stack([res.results[i]["out"] for i in range(NCORES)], axis=0)
    return outs.astype(np.float32), res


def kernel(**inputs):
    outs, _ = run_on_hw(inputs, trace=False)
    return outs


# revision 23
# speedup vs baseline: 1.3943x; 1.0070x over previous
"""Bass/Trainium2 kernel for nn_Encoders_6751688590031.

4-layer transformer encoder, d_model=64, H=8 heads, L=1024, dff=256, B=8.
Sharding: data-parallel over batch across 8 NeuronCores (1 batch element
per core); weights replicated. One AllReduce(max) per layer for the global
jnp.max(w) softmax normalization.

v3 design:
 - Logits: 4 heads packed per PE pass via 32-row tiling (K=9 row-tiles
   run concurrently). qaT/kaT hold 4 heads per group at 32-row pitch;
   row 32m+8 = ones (q) / padneg (k).
 - e^T v: v-stationary col-tiled matmuls (M=9 incl. ones column that
   accumulates per-head colsums -> s_h), PSUM-accumulated over q tiles.
   Output is an augmented feature-major attnT [128, L].
 - Wo and W2 applied TOKEN-major (lhsT = activation tile, rhs = weight),
   so no transposes / evacuations before the LNs; residual+bias adds are
   fused TTs from PSUM.
 - nz/s_h is folded into Wo's rhs before the AllReduce; the global 1/G
   is a post-AllReduce scalar fused into the residual add, so the Wo
   matmuls overlap the collective.
 - Per-head max of e: running bf16 TT-max folds (2x DVE) + final gpsimd
   XYZWC reduces (off the DVE critical path).
 - exp(): [128,1024] PSUM->SBUF bf16, strided out, 3-deep PSUM rotation.
 - Biases via augmented ones rows; grouped bn_stats; 2-step Newton rsqrt.
"""

import os
import sys

import numpy as np

for _p in (
    "/root/.axon_site",
    "/root/.axon_site/_ro/trn_rl_repo",
    "/root/.axon_site/_ro/pypackages",
    "/opt/trn_rl_repo",
):
    if os.path.isdir(_p) and _p not in sys.path:
        sys.path.append(_p)

import concourse.bass as bass
import concourse.bacc as bacc
import concourse.tile as tile
from concourse import mybir

F32 = mybir.dt.float32
F32R = mybir.dt.float32r
BF16 = mybir.dt.bfloat16

L = 1024          # sequence length
D = 64            # d_model
H = 8             # heads
DH = 8            # head dim
DFF = 256
NL = 4            # layers
P = 128           # partitions per token tile
NT = L // P       # 8 token tiles
NG = 2            # head groups (4 heads each)
MPG = 4           # heads per group
NCORES = 8
NEG_BIG = 1.0e9
LN_EPS = 1e-9

Act = mybir.ActivationFunctionType
Alu = mybir.AluOpType


def _r(ap):
    return ap.bitcast(F32R)


def build_bass():
    nc = bacc.Bacc(
        "TRN2", target_bir_lowering=False, debug=False, num_devices=NCORES
    )

    x_in = nc.declare_dram_parameter("x", [L, D], F32, isOutput=False)
    keepf = nc.declare_dram_parameter("keepf", [L], F32, isOutput=False)
    padneg = nc.declare_dram_parameter("padneg", [L], F32, isOutput=False)
    WqA = nc.declare_dram_parameter("WqA", [NL, NG, D + 1, P], F32, isOutput=False)
    WkA = nc.declare_dram_parameter("WkA", [NL, NG, D + 1, P], F32, isOutput=False)
    WvA = nc.declare_dram_parameter("WvA", [NL, NG, D + 1, 36], F32, isOutput=False)
    WoA = nc.declare_dram_parameter("WoA", [NL, NG, P, D], F32, isOutput=False)
    W1A = nc.declare_dram_parameter("W1A", [NL, 2, D + 1, P], F32, isOutput=False)
    W2 = nc.declare_dram_parameter("W2", [NL, 2, P, D], BF16, isOutput=False)
    LNV = nc.declare_dram_parameter("LNV", [NL, 6, D], F32, isOutput=False)
    B32 = nc.declare_dram_parameter("B32", [P, P], F32, isOutput=False)
    SELS = nc.declare_dram_parameter("SELS", [NG, P, MPG], F32, isOutput=False)
    out = nc.declare_dram_parameter("out", [L, D], F32, isOutput=True)

    dma = nc.sync.dma_start

    with tile.TileContext(nc) as tc:
        with (
            tc.tile_pool(name="const", bufs=1) as constp,
            tc.tile_pool(name="wpool", bufs=1) as wp,
            tc.tile_pool(name="lnp", bufs=2) as lnp,
            tc.tile_pool(name="persist", bufs=1) as pers,
            tc.tile_pool(name="acts", bufs=2) as acts,
            tc.tile_pool(name="epool", bufs=3) as epool,
            tc.tile_pool(name="rpool", bufs=2) as rpool,
            tc.tile_pool(name="stats", bufs=2) as stats,
            tc.tile_pool(name="psL", bufs=3, space="PSUM") as psL,
            tc.tile_pool(name="psA", bufs=1, space="PSUM") as psA,
            tc.tile_pool(name="dram", bufs=1, space="DRAM") as dramp,
        ):
            # ================= constants =================
            ones_t = constp.tile([P, P], F32, name="ones_t")
            nc.vector.memset(ones_t, 1.0)
            I128 = constp.tile([P, P], F32, name="I128")
            nc.gpsimd.affine_select(
                out=I128, in_=ones_t, pattern=[[-1, P]],
                compare_op=Alu.is_equal, fill=0.0, base=0, channel_multiplier=1,
            )
            I128R = constp.tile([P, P], F32R, name="I128R")
            nc.vector.tensor_copy(I128R, I128)
            ones_row8 = constp.tile([1, H], F32, name="ones_row8")
            nc.vector.memset(ones_row8, 1.0)
            ones_roww = constp.tile([1, L], F32, name="ones_roww")
            nc.vector.memset(ones_roww, 1.0)

            # zero psA banks once: never-written quadrant rows of the e^T v
            # accumulator then stay finite forever after
            pZI = psA.tile([P, L], F32, name="pZI", tag="A")
            nc.vector.memset(pZI, 0.0)

            def bcast_ap(dram_vec, parts, inner):
                return bass.AP(
                    tensor=dram_vec.tensor, offset=dram_vec.offset,
                    ap=[[0, parts]] + dram_vec.ap,
                )

            # keep broadcast [128, L]; rows 32m+8 stay 1.0 (ones rows)
            keep_b128 = constp.tile([P, L], F32, name="keep_b128")
            segs = [(0, 8), (9, 40), (41, 72), (73, 104), (105, 128)]
            for a, b in segs:
                dma(out=keep_b128[a:b, :], in_=bcast_ap(keepf[:], b - a, L))
            for m in range(MPG):
                nc.gpsimd.dma_start(
                    out=keep_b128[32 * m + 8: 32 * m + 9, :], in_=ones_roww[:])
            # keep broadcast [36, L] for v-aug; rows 9m+8 stay 1.0
            keep_b36 = constp.tile([36, L], F32, name="keep_b36")
            for m in range(MPG):
                dma(out=keep_b36[9 * m: 9 * m + 8, :], in_=bcast_ap(keepf[:], 8, L))
                nc.gpsimd.dma_start(
                    out=keep_b36[9 * m + 8: 9 * m + 9, :], in_=ones_roww[:])

            # token-major keep for nz and the final output mask
            keep_JP = constp.tile([NT, P], F32, name="keep_JP")
            dma(out=keep_JP, in_=keepf.rearrange("(j p) -> j p", p=P))
            pKA = psL.tile([P, L], F32, name="pKA", tag="L")
            nc.tensor.transpose(out=pKA[:, 0:NT], in_=keep_JP,
                                identity=I128[0:NT, 0:NT])
            keep_all = constp.tile([P, NT], F32, name="keep_all")
            nc.vector.tensor_copy(keep_all, pKA[:, 0:NT])
            keep_exp = constp.tile([P, NT, D], F32, name="keep_exp")
            for j in range(NT):
                nc.vector.tensor_scalar(
                    out=keep_exp[:, j, :], in0=ones_t[:, 0:D],
                    scalar1=keep_all[:, j: j + 1], scalar2=None, op0=Alu.mult,
                )
            keep_exp2 = keep_exp.rearrange("p j f -> p (j f)")

            # nz scalars: nz1 [1,1], nz8row [1,8], nzc [128,1]
            nzk = constp.tile([P, 1], F32, name="nzk")
            nc.vector.reduce_sum(out=nzk, in_=keep_all, axis=mybir.AxisListType.X)
            pNZ = psL.tile([P, L], F32, name="pNZ", tag="L")
            nc.tensor.matmul(pNZ[0:1, 0:1], ones_t[:, 0:1], nzk)
            nz1 = constp.tile([1, 1], F32, name="nz1")
            nc.vector.tensor_copy(nz1, pNZ[0:1, 0:1])
            pNZ8 = psL.tile([P, L], F32, name="pNZ8", tag="L")
            nc.tensor.matmul(pNZ8[0:H, 0:1], ones_row8, nz1)
            nc.tensor.matmul(pNZ8[:, 2:3], ones_t[0:1, :], nz1)
            nz8 = constp.tile([H, 1], F32, name="nz8")
            nc.vector.tensor_copy(nz8, pNZ8[0:H, 0:1])
            nzc = constp.tile([P, 1], F32, name="nzc")
            nc.vector.tensor_copy(nzc, pNZ8[:, 2:3])

            b32t = constp.tile([P, P], F32, name="b32t")
            dma(out=b32t, in_=B32[:, :])
            selt = []
            for g in range(NG):
                t2 = constp.tile([P, MPG], F32, name=f"selt{g}")
                dma(out=t2, in_=SELS[g])
                selt.append(t2)

            # ================= weights (consolidated DMAs) =================
            wqa_all = wp.tile([D + 1, NL, NG, P], F32R, name="wqa_all")
            dma(out=wqa_all, in_=_r(WqA[:, :, :, :].rearrange("l g r c -> r l g c")))
            wka_all = wp.tile([D + 1, NL, NG, P], F32R, name="wka_all")
            dma(out=wka_all, in_=_r(WkA[:, :, :, :].rearrange("l g r c -> r l g c")))
            wva_all = wp.tile([D + 1, NL, NG, 36], F32R, name="wva_all")
            dma(out=wva_all, in_=_r(WvA[:, :, :, :].rearrange("l g r c -> r l g c")))
            woa_all = wp.tile([P, NL, NG, D], F32, name="woa_all")
            dma(out=woa_all, in_=WoA[:, :, :, :].rearrange("l g r c -> r l g c"))
            w1a_all = wp.tile([D + 1, NL, 2, P], F32R, name="w1a_all")
            dma(out=w1a_all, in_=_r(W1A[:, :, :, :].rearrange("l i r c -> r l i c")))
            w2_all = wp.tile([P, NL, 2, D], BF16, name="w2_all")
            dma(out=w2_all, in_=W2[:, :, :, :].rearrange("l i r c -> r l i c"))

            # persistent augmented feature-major activations
            xTaug = pers.tile([D + 1, L], F32R, name="xTaug")
            nc.gpsimd.dma_start(out=xTaug[D: D + 1, :],
                                in_=ones_roww[:].bitcast(F32R))
            o1Taug = pers.tile([D + 1, L], F32R, name="o1Taug")
            nc.gpsimd.dma_start(out=o1Taug[D: D + 1, :],
                                in_=ones_roww[:].bitcast(F32R))
            qaT = [pers.tile([P, L], F32R, name=f"qaT{g}") for g in range(NG)]
            kaT = [pers.tile([P, L], F32R, name=f"kaT{g}") for g in range(NG)]

            # dummy AllReduce: absorbs core-start skew while setup runs
            dum_in = dramp.tile([1, 1], F32, name="dum_in", tag="dum_in")
            dum_out = dramp.tile([1, 1], F32, name="dum_out", tag="dum_out",
                                 addr_space="Shared")
            nc.gpsimd.dma_start(out=dum_in[:], in_=nz1)
            nc.gpsimd.collective_compute(
                "AllReduce", Alu.max,
                replica_groups=[list(range(NCORES))],
                ins=[dum_in.opt()], outs=[dum_out.opt()],
            )

            # ---- initial x load: token-major packed [128, (j,f)] ----
            x_all = acts.tile([P, NT * D], F32R, name="x_all", tag="x")
            dma(out=x_all.rearrange("p (j f) -> p j f", f=D),
                in_=_r(x_in.rearrange("(j p) f -> p j f", p=P)))

            def layernorm_z(z_all, gb, bb, x_out_name, x_out_tag, mask=None):
                zv = z_all.rearrange("p (j f) -> p j f", f=D)
                bn6 = stats.tile([P, NT, 6], F32, name=x_out_name + "_bn6",
                                 tag="bn6")
                mv = stats.tile([P, NT, 2], F32, name=x_out_name + "_mv",
                                tag="mv")
                for j in range(NT):
                    nc.vector.bn_stats(out=bn6[:, j, :], in_=zv[:, j, :])
                    nc.vector.bn_aggr(out=mv[:, j, :], in_=bn6[:, j, :])
                rstd = stats.tile([P, NT], F32, name=x_out_name + "_rstd",
                                  tag="rstd")
                vv = stats.tile([P, NT], F32, name=x_out_name + "_vv", tag="vv")
                nc.vector.tensor_scalar(out=vv, in0=mv[:, :, 1],
                                        scalar1=LN_EPS, scalar2=None,
                                        op0=Alu.add)
                iv = vv.bitcast(mybir.dt.int32)
                ir = rstd.bitcast(mybir.dt.int32)
                nc.vector.tensor_scalar(out=ir, in0=iv, scalar1=1,
                                        scalar2=None,
                                        op0=Alu.logical_shift_right)
                nc.vector.tensor_scalar(out=ir, in0=ir, scalar1=-1,
                                        scalar2=0x5F3759DF,
                                        op0=Alu.mult, op1=Alu.add)
                tq = stats.tile([P, NT], F32, name=x_out_name + "_tq", tag="tq")
                for _ in range(1):
                    nc.vector.tensor_mul(tq, rstd, rstd)
                    nc.vector.tensor_mul(tq, tq, vv)
                    nc.vector.tensor_scalar(out=tq, in0=tq, scalar1=-0.5,
                                            scalar2=1.5,
                                            op0=Alu.mult, op1=Alu.add)
                    nc.vector.tensor_mul(rstd, rstd, tq)
                o = acts.tile([P, NT * D], F32R, name=x_out_name, tag=x_out_tag)
                mb = mv[:, :, 0:1]
                mean_b = bass.AP(tensor=mb.tensor, offset=mb.offset,
                                 ap=[mb.ap[0], mb.ap[1], [0, D]])
                rb = rstd[:, :]
                rstd_b = bass.AP(tensor=rb.tensor, offset=rb.offset,
                                 ap=[rb.ap[0], rb.ap[1], [0, D]])
                ov = o.rearrange("p (j f) -> p j f", f=D)
                nc.vector.tensor_tensor(ov, zv, mean_b, op=Alu.subtract)
                nc.vector.tensor_tensor(ov, ov, rstd_b, op=Alu.mult)
                nc.vector.tensor_mul(o, o, gb)
                nc.vector.tensor_add(o, o, bb)
                if mask is not None:
                    nc.vector.tensor_mul(o, o, mask)
                return o

            for l in range(NL):
                # per-layer LN/bias vectors broadcast: [128, 6, NT, D]
                # order: g1, be1, g2, be2, bo, b2
                lnv = lnp.tile([P, 6, NT, D], F32, name=f"lnv{l}", tag="lnv")
                for v in range(6):
                    lv = LNV[l, v]
                    dma(out=lnv[:, v], in_=bass.AP(
                        tensor=lv.tensor, offset=lv.offset,
                        ap=[[0, P], [0, NT], [1, D]]))
                g1b = lnv[:, 0].rearrange("p j f -> p (j f)")
                be1b = lnv[:, 1].rearrange("p j f -> p (j f)")
                g2b = lnv[:, 2].rearrange("p j f -> p (j f)")
                be2b = lnv[:, 3].rearrange("p j f -> p (j f)")
                bo_b = lnv[:, 4].rearrange("p j f -> p (j f)")
                b2_b = lnv[:, 5].rearrange("p j f -> p (j f)")

                # ======== xTaug (feature-major + ones row) ========
                pX = psA.tile([P, L], F32, name=f"pX{l}", tag="A")
                for j in range(NT):
                    nc.tensor.transpose(
                        out=_r(pX[0:D, j * P: (j + 1) * P]),
                        in_=x_all[:, j * D: (j + 1) * D],
                        identity=I128R,
                    )
                for half in range(2):
                    nc.vector.tensor_copy(
                        xTaug[0:D, half * 512: (half + 1) * 512],
                        pX[0:D, half * 512: (half + 1) * 512],
                    )

                # ======== attention (QKV inlined per group) ========
                attnTa = []
                woaS_l = []
                cc_outs = []
                for g in range(NG):
                    pQ = psL.tile([P, L], F32, name=f"pQ{l}_{g}", tag="L")
                    for hf in range(2):
                        nc.tensor.matmul(pQ[:, hf * 512: (hf + 1) * 512],
                                         wqa_all[:, l, g, :],
                                         xTaug[:, hf * 512: (hf + 1) * 512])
                    nc.vector.tensor_tensor(qaT[g], pQ, keep_b128, op=Alu.mult)

                    pK = psL.tile([P, L], F32, name=f"pK{l}_{g}", tag="L")
                    for hf in range(2):
                        nc.tensor.matmul(pK[:, hf * 512: (hf + 1) * 512],
                                         wka_all[:, l, g, :],
                                         xTaug[:, hf * 512: (hf + 1) * 512])
                    if g == 0:
                        nc.scalar.activation(out=kaT[g], in_=pK, func=Act.Copy)
                    else:
                        nc.vector.tensor_copy(kaT[g], pK)
                    for m in range(MPG):
                        nc.gpsimd.dma_start(
                            out=kaT[g][32 * m + 8: 32 * m + 9, :],
                            in_=padneg.rearrange("(o n) -> o n", o=1).bitcast(F32R))

                    pV = psL.tile([P, L], F32, name=f"pV{l}_{g}", tag="L")
                    for hf in range(2):
                        nc.tensor.matmul(pV[0:36, hf * 512: (hf + 1) * 512],
                                         wva_all[:, l, g, :],
                                         xTaug[:, hf * 512: (hf + 1) * 512])
                    vTx = acts.tile([36, L], F32R, name=f"vTx{l}_{g}", tag="vTx")
                    nc.vector.tensor_tensor(vTx, pV[0:36, :], keep_b36, op=Alu.mult)
                    # token-major v-aug bf16 [128, NT, 36]
                    pVt = psL.tile([P, L], F32, name=f"pVt{l}_{g}", tag="L")
                    for j in range(NT):
                        nc.tensor.transpose(
                            out=_r(pVt[:, j * P: j * P + 36]),
                            in_=vTx[:, j * P: (j + 1) * P],
                            identity=I128R[0:36, 0:36],
                        )
                    vt_g = acts.tile([P, NT, 36], BF16, name=f"vta{l}_{g}",
                                     tag=f"vta{g}")
                    nc.vector.tensor_copy(
                        vt_g, pVt.rearrange("p (j c) -> p j c", c=P)[:, :, 0:36])

                    R_g = rpool.tile([P, MPG, L], BF16, name=f"R{l}_{g}", tag="R")
                    pAcc = psA.tile([P, L], F32, name=f"pAcc{l}_{g}", tag="A")
                    for j in range(NT):
                        et = epool.tile([P, MPG, L], BF16, name=f"e{l}_{g}_{j}",
                                        tag="e")
                        for kh in range(2):
                            for mp in range(2):
                                pL = psL.tile([P, L], F32,
                                              name=f"pL{l}_{g}_{j}_{kh}_{mp}",
                                              tag="L")
                                for mh in range(2):
                                    m = 2 * mp + mh
                                    rb = 32 * m
                                    nc.tensor.matmul(
                                        pL[:, mh * 512: (mh + 1) * 512],
                                        qaT[g][rb: rb + 9, j * P: (j + 1) * P],
                                        kaT[g][rb: rb + 9,
                                               kh * 512: (kh + 1) * 512],
                                        tile_position=(rb, 0),
                                    )
                                nc.scalar.activation(
                                    out=et[:, 2 * mp: 2 * mp + 2,
                                           kh * 512: (kh + 1) * 512],
                                    in_=pL.rearrange("p (m k) -> p m k", k=512),
                                    func=Act.Exp,
                                )
                        for kh in range(2):
                            for m in range(MPG):
                                nc.tensor.matmul(
                                    pAcc[32 * m: 32 * m + 9,
                                         kh * 512: (kh + 1) * 512],
                                    vt_g[:, j, 9 * m: 9 * m + 9],
                                    et[:, m, kh * 512: (kh + 1) * 512],
                                    start=(j == 0), stop=(j == NT - 1),
                                    skip_group_check=True,
                                    tile_position=(0, 32 * m),
                                )
                        if j == 0:
                            nc.vector.tensor_copy(R_g, et)
                        else:
                            nc.vector.tensor_tensor(R_g, R_g, et, op=Alu.max)
                    sm = stats.tile([P, 1], F32, name=f"sms{l}_{g}", tag=f"sms{g}")
                    nc.vector.reduce_sum(out=sm, in_=pAcc,
                                         axis=mybir.AxisListType.X)
                    at = acts.tile([P, L], BF16, name=f"attnTa{l}_{g}",
                                   tag=f"at{g}")
                    nc.vector.tensor_copy(at, pAcc)
                    attnTa.append(at)

                    # nz / s_h broadcast over each 32-row block -> scale Wo rhs
                    pCp = psL.tile([P, L], F32, name=f"pCp{l}_{g}", tag="L")
                    nc.tensor.matmul(pCp[:, 0:1], b32t, sm)
                    nc.tensor.matmul(pCp[0:MPG, 1:2], selt[g], sm)
                    rsc = stats.tile([P, 1], F32, name=f"rsc{l}_{g}",
                                     tag=f"rsc{g}")
                    nc.vector.reciprocal(out=rsc, in_=pCp[:, 0:1])
                    caf = stats.tile([P, 1], F32, name=f"caf{l}_{g}",
                                     tag=f"caf{g}")
                    nc.vector.tensor_mul(caf, nzc, rsc)
                    woaS = stats.tile([P, D], BF16, name=f"woaS{l}_{g}",
                                      tag=f"woaS{g}")
                    nc.vector.tensor_scalar(out=woaS, in0=woa_all[:, l, g, :],
                                            scalar1=caf, scalar2=None,
                                            op0=Alu.mult)
                    woaS_l.append(woaS)

                    # per-grp t4 = nz*mx/s and its own AllReduce; grp0's
                    # collective is fully hidden under grp1's attention
                    mxs = stats.tile([P, MPG], F32R, name=f"mxs{l}_{g}",
                                     tag=f"mxs{g}")
                    nc.vector.tensor_reduce(
                        out=mxs, in_=R_g, axis=mybir.AxisListType.X, op=Alu.max)
                    pS = psL.tile([P, L], F32, name=f"pS{l}_{g}", tag="L")
                    nc.tensor.transpose(out=_r(pS[0:MPG, 0:P]), in_=mxs,
                                        identity=I128R)
                    mx4 = stats.tile([MPG, 1], F32, name=f"mx4{l}_{g}",
                                     tag=f"mx4{g}")
                    nc.vector.reduce_max(out=mx4, in_=pS[0:MPG, 0:P],
                                         axis=mybir.AxisListType.X)
                    rs4 = stats.tile([MPG, 1], F32, name=f"rs4_{l}_{g}",
                                     tag=f"rs4{g}")
                    nc.vector.reciprocal(out=rs4, in_=pCp[0:MPG, 1:2])
                    t4 = stats.tile([MPG, 1], F32, name=f"t4_{l}_{g}",
                                    tag=f"t4{g}")
                    nc.vector.scalar_tensor_tensor(
                        out=t4, in0=mx4, scalar=nz8[0:MPG, :], in1=rs4,
                        op0=Alu.mult, op1=Alu.mult)
                    cc_in = dramp.tile([MPG, 1], F32, name=f"cc_in{l}_{g}",
                                       tag=f"cc_in{l}_{g}")
                    cc_out = dramp.tile([MPG, 1], F32, name=f"cc_out{l}_{g}",
                                        tag=f"cc_out{l}_{g}",
                                        addr_space="Shared")
                    nc.gpsimd.dma_start(out=cc_in[:], in_=t4)
                    nc.gpsimd.collective_compute(
                        "AllReduce", Alu.max,
                        replica_groups=[list(range(NCORES))],
                        ins=[cc_in.opt()], outs=[cc_out.opt()],
                    )
                    cc_outs.append(cc_out)

                Grow = stats.tile([1, H], F32, name=f"Grow{l}", tag="Grow")
                for g in range(NG):
                    nc.gpsimd.dma_start(
                        out=Grow[:, MPG * g: MPG * (g + 1)],
                        in_=cc_outs[g][:].rearrange("a b -> b a"))
                G = stats.tile([1, 1], F32, name=f"G{l}", tag=f"G{l}")
                nc.vector.reduce_max(out=G, in_=Grow, axis=mybir.AxisListType.X)

                # xb = x + bo (independent of G; overlaps the collective)
                xb = acts.tile([P, NT * D], F32, name=f"xb{l}", tag="xb")
                nc.vector.tensor_add(xb, x_all, bo_b)

                # ======== Wo token-major (overlaps the AllReduce) ========
                pZ1 = psA.tile([P, L], F32, name=f"pZ1{l}", tag="A")
                for j in range(NT):
                    for g in range(NG):
                        nc.tensor.matmul(
                            pZ1[:, j * D: (j + 1) * D],
                            attnTa[g][:, j * P: (j + 1) * P],
                            woaS_l[g],
                            start=(g == 0), stop=(g == NG - 1),
                            skip_group_check=True,
                        )

                # 1/G broadcast [128,1]
                pRG = psL.tile([P, L], F32, name=f"pRG{l}", tag="L")
                nc.tensor.matmul(pRG[:, 0:1], ones_t[0:1, :], G)
                rGa = stats.tile([P, 1], F32, name=f"rGa{l}", tag="rGa")
                nc.vector.reciprocal(out=rGa, in_=pRG[:, 0:1])

                # z = attn/G + x + bo, then LN1
                z1 = acts.tile([P, NT * D], F32, name=f"z1_{l}", tag="z1")
                nc.vector.scalar_tensor_tensor(
                    out=z1, in0=pZ1[:, 0: NT * D], scalar=rGa,
                    in1=xb, op0=Alu.mult, op1=Alu.add,
                )
                out1_all = layernorm_z(z1, g1b, be1b, f"out1_{l}", "out1")

                # ======== o1Taug + FFN ========
                pO = psA.tile([P, L], F32, name=f"pO{l}", tag="A")
                for j in range(NT):
                    nc.tensor.transpose(
                        out=_r(pO[0:D, j * P: (j + 1) * P]),
                        in_=out1_all[:, j * D: (j + 1) * D],
                        identity=I128R,
                    )
                for half in range(2):
                    nc.vector.tensor_copy(
                        o1Taug[0:D, half * 512: (half + 1) * 512],
                        pO[0:D, half * 512: (half + 1) * 512],
                    )

                h1 = []
                for i in range(2):
                    pH = psL.tile([P, L], F32, name=f"pH{l}_{i}", tag="L")
                    for hf in range(2):
                        nc.tensor.matmul(pH[:, hf * 512: (hf + 1) * 512],
                                         w1a_all[:, l, i, :],
                                         o1Taug[:, hf * 512: (hf + 1) * 512])
                    h1x = acts.tile([P, L], BF16, name=f"h1_{l}_{i}",
                                    tag=f"h1_{i}")
                    nc.scalar.activation(out=h1x, in_=pH, func=Act.Relu)
                    h1.append(h1x)

                # xb2 = out1 + b2 (ready before W2 lands)
                xb2 = acts.tile([P, NT * D], F32, name=f"xb2{l}", tag="xb2")
                nc.vector.tensor_add(xb2, out1_all, b2_b)

                # W2 token-major
                pZ2 = psA.tile([P, L], F32, name=f"pZ2{l}", tag="A")
                for j in range(NT):
                    for i in range(2):
                        nc.tensor.matmul(
                            pZ2[:, j * D: (j + 1) * D],
                            h1[i][:, j * P: (j + 1) * P],
                            w2_all[:, l, i, :],
                            start=(i == 0), stop=(i == 1),
                            skip_group_check=True,
                        )
                z2 = acts.tile([P, NT * D], F32, name=f"z2_{l}", tag="z2")
                nc.vector.tensor_add(z2, pZ2[:, 0: NT * D], xb2)

                x_all = layernorm_z(z2, g2b, be2b, f"x_next_{l}", "x",
                                    mask=keep_exp2 if l == NL - 1 else None)

            dma(out=out.rearrange("(j p) f -> p j f", p=P),
                in_=x_all.rearrange("p (j f) -> p j f", f=D).bitcast(F32))

    return nc


_NC_CACHE = None


def _get_nc():
    global _NC_CACHE
    if _NC_CACHE is None:
        _NC_CACHE = build_bass()
    return _NC_CACHE


def _make_in_maps(inputs):
    x = np.asarray(inputs["x"], np.float32)
    protok = np.asarray(inputs["protok"])
    B = x.shape[0]
    keep = (protok != 0).astype(np.float32)
    padneg = (keep - 1.0) * NEG_BIG

    Wq = np.asarray(inputs["Wq"], np.float32)
    Wk = np.asarray(inputs["Wk"], np.float32)
    Wv = np.asarray(inputs["Wv"], np.float32)
    Wo = np.asarray(inputs["Wo"], np.float32)
    bq = np.asarray(inputs["bq"], np.float32)
    bk = np.asarray(inputs["bk"], np.float32)
    bv = np.asarray(inputs["bv"], np.float32)
    W1 = np.asarray(inputs["W1"], np.float32)
    b1 = np.asarray(inputs["b1"], np.float32)
    W2in = np.asarray(inputs["W2"], np.float32)

    WqA = np.zeros((NL, NG, D + 1, P), np.float32)
    WkA = np.zeros((NL, NG, D + 1, P), np.float32)
    WvA = np.zeros((NL, NG, D + 1, 36), np.float32)
    WoA = np.zeros((NL, NG, P, D), np.float32)
    for g in range(NG):
        for m in range(MPG):
            h = MPG * g + m
            cols = slice(DH * h, DH * (h + 1))
            WqA[:, g, 0:D, 32 * m: 32 * m + DH] = Wq[:, :, cols]
            WqA[:, g, D, 32 * m: 32 * m + DH] = bq[:, cols]
            WqA[:, g, D, 32 * m + 8] = 1.0
            WkA[:, g, 0:D, 32 * m: 32 * m + DH] = Wk[:, :, cols]
            WkA[:, g, D, 32 * m: 32 * m + DH] = bk[:, cols]
            WvA[:, g, 0:D, 9 * m: 9 * m + DH] = Wv[:, :, cols]
            WvA[:, g, D, 9 * m: 9 * m + DH] = bv[:, cols]
            WvA[:, g, D, 9 * m + 8] = 1.0
            WoA[:, g, 32 * m: 32 * m + DH, :] = Wo[:, cols, :]

    W1A = np.zeros((NL, 2, D + 1, P), np.float32)
    for i in range(2):
        W1A[:, i, 0:D, :] = W1[:, :, i * P: (i + 1) * P]
        W1A[:, i, D, :] = b1[:, i * P: (i + 1) * P]
    import ml_dtypes
    W2A = np.ascontiguousarray(
        W2in.reshape(NL, 2, P, D).astype(ml_dtypes.bfloat16))

    LNVc = np.stack([
        np.asarray(inputs["g1"], np.float32),
        np.asarray(inputs["be1"], np.float32),
        np.asarray(inputs["g2"], np.float32),
        np.asarray(inputs["be2"], np.float32),
        np.asarray(inputs["bo"], np.float32),
        np.asarray(inputs["b2"], np.float32),
    ], axis=1)  # [NL, 6, D]

    B32c = np.zeros((P, P), np.float32)
    for r in range(P):
        B32c[32 * (r // 32) + 8, r] = 1.0
    SELSc = np.zeros((NG, P, MPG), np.float32)
    for g in range(NG):
        for m in range(MPG):
            SELSc[g, 32 * m + 8, m] = 1.0

    shared = dict(
        WqA=WqA, WkA=WkA, WvA=WvA, WoA=WoA,
        W1A=W1A, W2=W2A, LNV=np.ascontiguousarray(LNVc),
        B32=B32c, SELS=SELSc,
    )
    in_maps = []
    for i in range(NCORES):
        b = i % B
        in_maps.append(dict(
            x=np.ascontiguousarray(x[b]),
            keepf=np.ascontiguousarray(keep[b]),
            padneg=np.ascontiguousarray(padneg[b]),
            **shared,
        ))
    return in_maps


def run_on_hw(inputs, trace=False, **kwargs):
    from concourse.bass_utils import run_bass_kernel_spmd

    nc = _get_nc()
    if not nc.is_finalized():
        nc.finalize()
    in_maps = _make_in_maps(inputs)
    res = run_bass_kernel_spmd(nc, in_maps, list(range(NCORES)), trace=trace, **kwargs)
    outs = np.stack([res.results[i]["out"] for i in range(NCORES)], axis=0)
    return outs.astype(np.float32), res


def kernel(**inputs):
    outs, _ = run_on_hw(inputs, trace=False)
    return outs
